# revision 2
# baseline (speedup 1.0000x reference)
"""Bidirectional attention kernel for Trainium2 (8 NeuronCores, data-parallel
over batch) with host-side mask compaction and per-slot adaptive shapes.

~50% of rows on each side are padding (mask True). Masked rows contribute
exp(MASK_FILL - max) ~ 0 to the softmax sums, and their output rows are
zeroed. So we gather the keep rows on the host, and run dense bidirectional
attention on compacted [n1, n2] similarity slabs. Zero-padded rows
self-mask: v=0 -> sim=0 -> exp(0-88) ~ 6e-39 ~ 0 in bf16, and their
ones-column entry is 0 so they don't touch denominators.

The 64 batches are grouped into 8 program slots of 8 (one batch per core
per slot, SPMD) so that batches with the same chunk shape share a slot:
slot j is compiled for (PCL_j, PCM_j) 128-chunks = the max keep counts in
its group. Groups are formed per exact chunk-class to minimize the summed
chunk products (PE work is ~ PCL*PCM).

Math (per slot, compacted):
  sim[l, m] = v1c[l] . v2c[m]                    (fp16 matmuls, [l,m] layout)
  E[l, m]   = exp(sim - C)  (ACT, accum_out -> S_a[l] = sum_m E[l,m])
  Ea[m, l]  = transpose(E)  (PE transpose + DVE copy psum->sbuf)
  u1T[d, l] = sum_m v2e[m, d] Ea[m, l]   (v2-stationary matmuls, PSUM accum
              over m-chunks; drained by plain DVE copies, UNNORMALIZED)
  u2[m, :]  = sum_l E[l, m] v1e[l, :]    (ones col in v1e -> S_b at col D;
              drained UNNORMALIZED by ACT copies)
  host: attended_v1[l] = u1T[:, l] / S_a[l]   (S_a DMA'd out per slot)
        attended_v2[m] = u2[m, 0:D] / u2[m, D]

C is a fixed softmax shift (exp(x-C) instead of exp(x-max)): mathematically
identical softmax, safe because |sim| <~ 91 << 176 and underflow terms are
negligible relative to row sums.
"""
import sys
import types

import ml_dtypes
import numpy as np
from contextlib import ExitStack


def _install_axon_hooks_shim():
    """The image's antenv package lacks the optional axon_hooks module that
    the axon boot shim uses to register the NTFF profiling hook (it degrades
    silently without it). Provide it and redo the registration the boot shim
    skipped, so trace=True works."""
    if "antenv.axon_hooks" not in sys.modules:
        mod = types.ModuleType("antenv.axon_hooks")
        mod._hook = None

        def set_axon_ntff_profile_hook(hook):
            mod._hook = hook

        def get_axon_ntff_profile_hook():
            return mod._hook

        mod.set_axon_ntff_profile_hook = set_axon_ntff_profile_hook
        mod.get_axon_ntff_profile_hook = get_axon_ntff_profile_hook
        sys.modules["antenv.axon_hooks"] = mod
        try:
            import antenv

            antenv.axon_hooks = mod
        except ImportError:
            pass
    mod = sys.modules["antenv.axon_hooks"]
    if getattr(mod, "_hook", None) is None:
        try:
            from trn_agent_boot.trn_boot import _ntff_profile_via_ctypes

            mod._hook = _ntff_profile_via_ctypes("/opt/axon/libaxon_pjrt.so")
        except Exception:
            pass


_install_axon_hooks_shim()

import concourse.bacc as bacc
import concourse.mybir as mybir
import concourse.tile as tile
from concourse.bass_utils import run_bass_kernel_spmd

F32 = mybir.dt.float32
BF16 = mybir.dt.bfloat16
FP16 = mybir.dt.float16
AF = mybir.ActivationFunctionType
ALU = mybir.AluOpType

B, L, D = 64, 1024, 256
NCORES = 8
BPC = B // NCORES          # batches per core == number of program slots
C_SHIFT = np.float32(88.0)
EW = D + 2  # v1e free width: col 256 = ones (S_b denominator), col 257 = 0
# pad (even free-dim count for the ISA)


def _even_ceil(x):
    return int(x + (x & 1))


def plan_slots(n1, n2):
    """Group the 64 batches into BPC slots of NCORES batches with matching
    chunk shapes. Returns (groups, specs): groups[j] = list of NCORES batch
    ids (core i takes groups[j][i]); specs[j] = dict of per-slot shapes."""
    cl = np.maximum(1, -(-n1 // 128)).astype(int)
    cm = np.maximum(1, -(-n2 // 128)).astype(int)
    classes = {}
    for b in range(B):
        classes.setdefault((int(cl[b]), int(cm[b])), []).append(b)
    groups = []
    leftovers = []
    for key in sorted(classes, key=lambda k: (-k[0] * k[1], -k[0])):
        lst = classes[key]
        while len(lst) >= NCORES:
            groups.append(lst[:NCORES])
            lst = lst[NCORES:]
        leftovers.extend(lst)
    leftovers.sort(key=lambda b: (-int(cl[b] * cm[b]), -int(cl[b])))
    while leftovers:
        groups.append(leftovers[:NCORES])
        leftovers = leftovers[NCORES:]
    assert len(groups) == BPC
    # order: smallest slot first (fast pipeline fill), second-smallest last
    # (short attend-A tail), rest in the middle
    groups.sort(key=lambda g: max(cl[b] * cm[b] for b in g))
    groups = [groups[0]] + groups[2:] + [groups[1]]
    specs = []
    for g in groups:
        PCL = int(max(cl[b] for b in g))
        PCM = int(max(cm[b] for b in g))
        mx1 = int(max(n1[b] for b in g))
        mx2 = int(max(n2[b] for b in g))
        specs.append(dict(
            PCL=PCL, PCM=PCM,
            Ppl=PCL * 128, Ppm=PCM * 128,
            Pms=min(PCM * 128, _even_ceil(mx2)),   # sim/exp m free width
            Pls=min(PCL * 128, _even_ceil(mx1)),   # attend-A l free width
            # transpose out width for the last l-chunk
            wl=min(128, _even_ceil(mx1 - (PCL - 1) * 128)),
        ))
    return groups, specs


def _lp_pieces(Pls):
    """Split [0, Pls) into <=512-wide pieces (one PSUM bank each)."""
    out = []
    o = 0
    while o < Pls:
        w = min(512, Pls - o)
        out.append((o, w))
        o += w
    return out


def build_nc(specs):
    nc = bacc.Bacc("TRN2", target_bir_lowering=False, debug=False)

    v1t, v2t, v1e, v2e, o1T, out2, sa_d = [], [], [], [], [], [], []
    for j, sp in enumerate(specs):
        Ppl, Ppm, PCL, PCM, Pls = sp["Ppl"], sp["Ppm"], sp["PCL"], sp["PCM"], sp["Pls"]
        v1t.append(nc.dram_tensor(f"v1t{j}", [128, 2 * Ppl], FP16, kind="ExternalInput").ap())
        v2t.append(nc.dram_tensor(f"v2t{j}", [128, 2 * Ppm], FP16, kind="ExternalInput").ap())
        v1e.append(nc.dram_tensor(f"v1e{j}", [128, PCL, EW], BF16, kind="ExternalInput").ap())
        v2e.append(nc.dram_tensor(f"v2e{j}", [128, PCM, D], BF16, kind="ExternalInput").ap())
        o1T.append(nc.dram_tensor(f"o1T{j}", [128, 2 * Pls], BF16, kind="ExternalOutput").ap())
        out2.append(nc.dram_tensor(f"out2_{j}", [128, PCM, EW], BF16, kind="ExternalOutput").ap())
        sa_d.append(nc.dram_tensor(f"sa{j}", [128, PCL], F32, kind="ExternalOutput").ap())
    ident_d = nc.dram_tensor("ident", [128, 128], BF16, kind="ExternalInput").ap()
    cbias_d = nc.dram_tensor("cbias", [128, 1], F32, kind="ExternalInput").ap()

    with tile.TileContext(nc) as tc, ExitStack() as ctx:
        const_pool = ctx.enter_context(tc.tile_pool(name="const", bufs=1))
        in_pool = ctx.enter_context(tc.tile_pool(name="inp", bufs=2))
        e_pool = ctx.enter_context(tc.tile_pool(name="epool", bufs=1))
        ebpool = ctx.enter_context(tc.tile_pool(name="ebpool", bufs=2))
        out_pool = ctx.enter_context(tc.tile_pool(name="outp", bufs=2))
        sa_pool = ctx.enter_context(tc.tile_pool(name="sa", bufs=2))
        psb_pool = ctx.enter_context(tc.tile_pool(name="psb", bufs=2, space="PSUM"))
        # psT (transpose, bf16 <=1280B) and psA (attend-A accum, f32 <=2048B)
        # share one pool: their live ranges are phase-disjoint within a slot.
        pst_pool = ctx.enter_context(tc.tile_pool(name="pst", bufs=2, space="PSUM"))
        pso_pool = ctx.enter_context(tc.tile_pool(name="pso", bufs=2, space="PSUM"))

        ident = const_pool.tile([128, 128], BF16)
        nc.sync.dma_start(ident[:], ident_d)

        # The Pms trim leaves columns [Pms, Ppm) of each Eb row-chunk
        # unwritten; the transposes feed those cells into the attend-A
        # contraction where a NaN bit-pattern from uninitialized SBUF would
        # poison valid rows (0 * NaN = NaN). Zero the full region once:
        # every later write to it is finite exp output.
        eb_max = max(sp["PCL"] * sp["Ppm"] for sp in specs)
        for _ in range(2):
            eb0 = ebpool.tile([128, eb_max], BF16, tag="Eb")
            nc.gpsimd.memset(eb0[:], 0.0)
        cbias = const_pool.tile([128, 1], F32)
        nc.sync.dma_start(cbias[:], cbias_d)

        # PE warmup: dummy transposes while the first slot's input DMAs
        # stream in, so the HAM clock-gate is at 2.4 GHz when real matmuls
        # start.
        warm = pst_pool.tile([128, 512], BF16, tag="psT")
        for w in range(48):
            nc.tensor.transpose(warm[:, (w % 4) * 128:(w % 4 + 1) * 128], ident[:], ident[:])

        # attend-A of slot j-1 is interleaved into slot j's sim phase so sim
        # matmuls absorb the psA-recycle latency between attend-A units.
        prev = None  # (Ea_sb, v2e_sb, out1T_sb, spec, j-1)

        def attend_a_unit(u, st):
            """One (d-half, l-piece) of attend-A for slot jp: accumulate
            over all m-chunks into a 1-bank PSUM tile, then plain-copy out
            (unnormalized; host divides by S_a)."""
            dh, (lp0, lpw) = u
            Ea_p, v2e_p, out1T_p, spp, jp = st
            Pls_p = spp["Pls"]
            psA = pst_pool.tile([128, lpw], F32, tag="psT")
            for mc in range(spp["PCM"]):
                nc.tensor.matmul(
                    psA[:],
                    v2e_p[:, mc * D + dh * 128: mc * D + (dh + 1) * 128],
                    Ea_p[:, mc * Pls_p + lp0: mc * Pls_p + lp0 + lpw],
                    start=(mc == 0),
                    stop=(mc == spp["PCM"] - 1),
                )
            dst0 = dh * Pls_p + lp0
            nc.vector.tensor_copy(out1T_p[:, dst0:dst0 + lpw], psA[:])
            nc.sync.dma_start(o1T[jp][:, dst0:dst0 + lpw], out1T_p[:, dst0:dst0 + lpw])

        for j, sp in enumerate(specs):
            PCL, PCM = sp["PCL"], sp["PCM"]
            Ppl, Ppm, Pms, Pls, wl = sp["Ppl"], sp["Ppm"], sp["Pms"], sp["Pls"], sp["wl"]

            v1t_sb = in_pool.tile([128, 2 * Ppl], FP16, tag="v1t")
            v2t_sb = in_pool.tile([128, 2 * Ppm], FP16, tag="v2t")
            v1e_sb = in_pool.tile([128, PCL * EW], BF16, tag="v1e")
            v2e_sb = in_pool.tile([128, PCM * D], BF16, tag="v2e")
            if j == 0:
                nc.sync.dma_start(v1t_sb[:, 0:Ppl], v1t[j][:, 0:Ppl])
                nc.sync.dma_start(v2t_sb[:, 0:Ppm], v2t[j][:, 0:Ppm])
                nc.sync.dma_start(v1t_sb[:, Ppl:2 * Ppl], v1t[j][:, Ppl:2 * Ppl])
                nc.sync.dma_start(v2t_sb[:, Ppm:2 * Ppm], v2t[j][:, Ppm:2 * Ppm])
            else:
                nc.sync.dma_start(v1t_sb[:], v1t[j])
                nc.sync.dma_start(v2t_sb[:], v2t[j])
            nc.sync.dma_start(v1e_sb[:, 0:PCL * EW], v1e[j].rearrange("p c j -> p (c j)"))
            nc.sync.dma_start(v2e_sb[:, 0:PCM * D], v2e[j].rearrange("p c j -> p (c j)"))

            Eb_sb = ebpool.tile([128, PCL * Ppm], BF16, tag="Eb")
            Ea_sb = e_pool.tile([128, PCM * Pls], BF16, tag="Ea")
            Sa_sb = sa_pool.tile([128, PCL], F32, tag="Sa")
            out1T_sb = out_pool.tile([128, 2 * Pls], BF16, tag="o1")
            out2_sb = out_pool.tile([128, PCM * EW], BF16, tag="o2")

            # ---- Phase 1: sim in [l, m] layout + fused shift/exp -> Eb
            # (accum_out gives S_a row sums for free; attend-A units of
            # slot j-1 interleaved between sim groups). Columns beyond Pms
            # of each Eb row-chunk are stale: they only feed discarded
            # output rows (m >= n2) or get multiplied by zero pad rows.
            units = [] if prev is None else [
                (dh, lp) for dh in (0, 1) for lp in _lp_pieces(prev[3]["Pls"])
            ]
            n_inter = min(len(units), max(PCL - 1, 0))
            ui = 0
            for lc in range(PCL):
                psB = psb_pool.tile([128, Ppm], F32, tag="psB")
                for k in range(2):
                    for mh in range(0, Pms, 512):
                        mw = min(512, Pms - mh)
                        nc.tensor.matmul(
                            psB[:, mh:mh + mw],
                            v1t_sb[:, k * Ppl + lc * 128: k * Ppl + (lc + 1) * 128],
                            v2t_sb[:, k * Ppm + mh: k * Ppm + mh + mw],
                            start=(k == 0),
                            stop=(k == 1),
                        )
                nc.scalar.activation(
                    Eb_sb[:, lc * Ppm: lc * Ppm + Pms],
                    psB[:, 0:Pms],
                    AF.Exp,
                    bias=cbias[:],
                    scale=1.0,
                    accum_out=Sa_sb[:, lc:lc + 1],
                )
                if prev is not None and 1 <= lc and ui < n_inter:
                    attend_a_unit(units[ui], prev)
                    ui += 1

            nc.sync.dma_start(sa_d[j][:], Sa_sb[:, 0:PCL])
            while ui < len(units):
                attend_a_unit(units[ui], prev)
                ui += 1

            # ---- Phase 2+3 interleaved: attend-B groups between transpose
            # groups so the PSUM->SBUF copies (DVE) overlap attend-B matmul
            # time on the PE instead of gating it. attend-B output (with the
            # S_b ones-column) is drained unnormalized by ACT copies.
            psT_cur = [None]
            g0 = (PCL + 1) // 2  # transposes emitted before the attend-B group

            def transp_half(mc, g):
                if g == 0:
                    psT_new = pst_pool.tile([128, Ppl], BF16, tag="psT")
                    psT_cur[0] = psT_new
                psT = psT_cur[0]
                lcs = range(g0) if g == 0 else range(g0, PCL)
                for lc in lcs:
                    w = wl if lc == PCL - 1 else 128
                    nc.tensor.transpose(
                        psT[:, lc * 128: lc * 128 + w],
                        Eb_sb[:, lc * Ppm + mc * 128: lc * Ppm + (mc + 1) * 128],
                        ident[:, 0:w],
                    )
                if g == 0:
                    return
                nc.vector.tensor_copy(Ea_sb[:, mc * Pls:(mc + 1) * Pls], psT[:, 0:Pls])

            for c in range(PCM):
                transp_half(c, 0)
                psO2 = pso_pool.tile([128, EW], F32, tag="psO")
                for k in range(PCL):
                    nc.tensor.matmul(
                        psO2[:],
                        Eb_sb[:, k * Ppm + c * 128: k * Ppm + (c + 1) * 128],
                        v1e_sb[:, k * EW:(k + 1) * EW],
                        start=(k == 0),
                        stop=(k == PCL - 1),
                    )
                nc.scalar.copy(out2_sb[:, c * EW:(c + 1) * EW], psO2[:])
                transp_half(c, 1)
            nc.sync.dma_start(out2[j].rearrange("p c j -> p (c j)"), out2_sb[:, 0:PCM * EW])

            prev = (Ea_sb, v2e_sb, out1T_sb, sp, j)

        for u in [(dh, lp) for dh in (0, 1) for lp in _lp_pieces(prev[3]["Pls"])]:
            attend_a_unit(u, prev)

    nc.compile()
    return nc


def _prep_slot_inputs(sp, v1b, n1b, v2b, n2b):
    """One batch -> the slot's input tensors. v1b/v2b [L, D] f32 full rows;
    n1b/n2b keep counts after compaction (rows [0:n) valid, rest zero)."""
    f32 = np.float32
    bf = ml_dtypes.bfloat16
    Ppl, Ppm, PCL, PCM = sp["Ppl"], sp["Ppm"], sp["PCL"], sp["PCM"]
    v1c = np.zeros((Ppl, D), f32)
    v1c[:n1b] = v1b[:n1b]
    v2c = np.zeros((Ppm, D), f32)
    v2c[:n2b] = v2b[:n2b]
    ones1 = np.zeros((Ppl, 1), f32)
    ones1[:n1b] = 1.0
    zeros = np.zeros((Ppl, 1), f32)
    v1e = np.concatenate([v1c, ones1, zeros], axis=1).reshape(PCL, 128, EW)
    v2e = v2c.reshape(PCM, 128, D)
    return {
        "v1t": np.ascontiguousarray(
            v1c.T.reshape(2, 128, Ppl).transpose(1, 0, 2).reshape(128, 2 * Ppl)
        ).astype(np.float16),
        "v2t": np.ascontiguousarray(
            v2c.T.reshape(2, 128, Ppm).transpose(1, 0, 2).reshape(128, 2 * Ppm)
        ).astype(np.float16),
        "v1e": np.ascontiguousarray(v1e.transpose(1, 0, 2)).astype(bf),
        "v2e": np.ascontiguousarray(v2e.transpose(1, 0, 2)).astype(bf),
    }


def run_on_hw(v1, v1_mask, v2, v2_mask, trace=False, nc=None, plan=None):
    i1s = [np.flatnonzero(~v1_mask[b]) for b in range(B)]
    i2s = [np.flatnonzero(~v2_mask[b]) for b in range(B)]
    n1 = np.array([len(i) for i in i1s])
    n2 = np.array([len(i) for i in i2s])
    if plan is None:
        plan = plan_slots(n1, n2)
    groups, specs = plan
    if nc is None:
        nc = build_nc(specs)
    in_maps = [{"ident": np.eye(128, dtype=ml_dtypes.bfloat16),
                "cbias": np.full((128, 1), -C_SHIFT, np.float32)} for _ in range(NCORES)]
    for j, (g, sp) in enumerate(zip(groups, specs)):
        for i, gb in enumerate(g):
            v1c = v1[gb][i1s[gb]]
            v2c = v2[gb][i2s[gb]]
            m = _prep_slot_inputs(sp, v1c, len(i1s[gb]), v2c, len(i2s[gb]))
            for nm, arr in m.items():
                in_maps[i][f"{nm}{j}"] = arr
    res = run_bass_kernel_spmd(nc, in_maps, core_ids=list(range(NCORES)), trace=trace)
    a1 = np.zeros((B, L, D), np.float32)
    a2 = np.zeros((B, L, D), np.float32)
    for j, (g, sp) in enumerate(zip(groups, specs)):
        Pls, PCM = sp["Pls"], sp["PCM"]
        for i, gb in enumerate(g):
            r = res.results[i]
            n1b, n2b = len(i1s[gb]), len(i2s[gb])
            # o1T [128(dpart), 2*Pls] -> [Pls, 256] unnormalized, / S_a
            u1 = r[f"o1T{j}"].reshape(128, 2, Pls).transpose(2, 1, 0).reshape(Pls, 2 * 128)
            sa = r[f"sa{j}"].T.reshape(-1)  # [Ppl]
            a1[gb, i1s[gb]] = (u1[:n1b].astype(np.float32)
                               / np.maximum(sa[:n1b, None], 1e-30))
            # out2 [128, PCM, EW] -> [Ppm, EW]; col D = S_b
            u2 = r[f"out2_{j}"].transpose(1, 0, 2).reshape(PCM * 128, EW)[:n2b]
            u2 = u2.astype(np.float32)
            a2[gb, i2s[gb]] = u2[:, 0:D] / np.maximum(u2[:, D:D + 1], 1e-30)
    return (a1, a2), res


def kernel(v1, v1_mask, v2, v2_mask):
    v1 = np.asarray(v1, np.float32)
    v2 = np.asarray(v2, np.float32)
    v1_mask = np.asarray(v1_mask)
    v2_mask = np.asarray(v2_mask)
    (a1, a2), _ = run_on_hw(v1, v1_mask, v2, v2_mask, trace=False)
    return a1, a2


# revision 4
# speedup vs baseline: 1.0226x; 1.0226x over previous
"""Bidirectional attention kernel for Trainium2 (8 NeuronCores, data-parallel
over batch) with host-side mask compaction and per-slot adaptive shapes.

~50% of rows on each side are padding (mask True). Masked rows contribute
exp(MASK_FILL - max) ~ 0 to the softmax sums, and their output rows are
zeroed. So we gather the keep rows on the host, and run dense bidirectional
attention on compacted [n1, n2] similarity slabs. Zero-padded rows
self-mask: v=0 -> sim=0 -> exp(0-88) ~ 6e-39 ~ 0 in bf16, and their
ones-column entry is 0 so they don't touch denominators.

The 64 batches are grouped into 8 program slots of 8 (one batch per core
per slot, SPMD) so that batches with the same chunk shape share a slot:
slot j is compiled for (PCL_j, PCM_j) 128-chunks = the max keep counts in
its group. Groups are formed per exact chunk-class to minimize the summed
chunk products (PE work is ~ PCL*PCM).

Math (per slot, compacted):
  sim[l, m] = v1c[l] . v2c[m]                    (fp16 matmuls, [l,m] layout)
  E[l, m]   = exp(sim - C)  (ACT, accum_out -> S_a[l] = sum_m E[l,m])
  Ea[m, l]  = transpose(E)  (PE transpose + DVE copy psum->sbuf)
  u1T[d, l] = sum_m v2e[m, d] Ea[m, l]   (v2-stationary matmuls, PSUM accum
              over m-chunks; drained by plain DVE copies, UNNORMALIZED)
  u2[m, :]  = sum_l E[l, m] v1e[l, :]    (ones col in v1e -> S_b at col D;
              drained UNNORMALIZED by ACT copies)
  host: attended_v1[l] = u1T[:, l] / S_a[l]   (S_a DMA'd out per slot)
        attended_v2[m] = u2[m, 0:D] / u2[m, D]

C is a fixed softmax shift (exp(x-C) instead of exp(x-max)): mathematically
identical softmax, safe because |sim| <~ 91 << 176 and underflow terms are
negligible relative to row sums.
"""
import sys
import types

import ml_dtypes
import numpy as np
from contextlib import ExitStack


def _install_axon_hooks_shim():
    """The image's antenv package lacks the optional axon_hooks module that
    the axon boot shim uses to register the NTFF profiling hook (it degrades
    silently without it). Provide it and redo the registration the boot shim
    skipped, so trace=True works."""
    if "antenv.axon_hooks" not in sys.modules:
        mod = types.ModuleType("antenv.axon_hooks")
        mod._hook = None

        def set_axon_ntff_profile_hook(hook):
            mod._hook = hook

        def get_axon_ntff_profile_hook():
            return mod._hook

        mod.set_axon_ntff_profile_hook = set_axon_ntff_profile_hook
        mod.get_axon_ntff_profile_hook = get_axon_ntff_profile_hook
        sys.modules["antenv.axon_hooks"] = mod
        try:
            import antenv

            antenv.axon_hooks = mod
        except ImportError:
            pass
    mod = sys.modules["antenv.axon_hooks"]
    if getattr(mod, "_hook", None) is None:
        try:
            from trn_agent_boot.trn_boot import _ntff_profile_via_ctypes

            mod._hook = _ntff_profile_via_ctypes("/opt/axon/libaxon_pjrt.so")
        except Exception:
            pass


_install_axon_hooks_shim()

import concourse.bacc as bacc
import concourse.mybir as mybir
import concourse.tile as tile
from concourse.bass_utils import run_bass_kernel_spmd

F32 = mybir.dt.float32
BF16 = mybir.dt.bfloat16
FP16 = mybir.dt.float16
AF = mybir.ActivationFunctionType
ALU = mybir.AluOpType

B, L, D = 64, 1024, 256
NCORES = 8
BPC = B // NCORES          # batches per core == number of program slots
C_SHIFT = np.float32(88.0)
EW = D + 2  # v1e free width: col 256 = ones (S_b denominator), col 257 = 0
# pad (even free-dim count for the ISA)


def _even_ceil(x):
    return int(x + (x & 1))


def plan_slots(n1, n2):
    """Group the 64 batches into BPC slots of NCORES batches with matching
    chunk shapes. Returns (groups, specs): groups[j] = list of NCORES batch
    ids (core i takes groups[j][i]); specs[j] = dict of per-slot shapes."""
    cl = np.maximum(1, -(-n1 // 128)).astype(int)
    cm = np.maximum(1, -(-n2 // 128)).astype(int)
    classes = {}
    for b in range(B):
        classes.setdefault((int(cl[b]), int(cm[b])), []).append(b)
    groups = []
    leftovers = []
    for key in sorted(classes, key=lambda k: (-k[0] * k[1], -k[0])):
        lst = classes[key]
        while len(lst) >= NCORES:
            groups.append(lst[:NCORES])
            lst = lst[NCORES:]
        leftovers.extend(lst)
    leftovers.sort(key=lambda b: (-int(cl[b] * cm[b]), -int(cl[b])))
    while leftovers:
        groups.append(leftovers[:NCORES])
        leftovers = leftovers[NCORES:]
    assert len(groups) == BPC
    # order: smallest slot first (fast pipeline fill), second-smallest last
    # (short attend-A tail), rest in the middle
    groups.sort(key=lambda g: max(cl[b] * cm[b] for b in g))
    groups = [groups[0]] + groups[2:] + [groups[1]]
    specs = []
    for g in groups:
        PCL = int(max(cl[b] for b in g))
        PCM = int(max(cm[b] for b in g))
        mx1 = int(max(n1[b] for b in g))
        mx2 = int(max(n2[b] for b in g))
        specs.append(dict(
            PCL=PCL, PCM=PCM,
            Ppl=PCL * 128, Ppm=PCM * 128,
            Pms=min(PCM * 128, _even_ceil(mx2)),   # sim/exp m free width
            Pls=min(PCL * 128, _even_ceil(mx1)),   # attend-A l free width
            # transpose out width for the last l-chunk
            wl=min(128, _even_ceil(mx1 - (PCL - 1) * 128)),
        ))
    return groups, specs


def _lp_pieces(Pls):
    """Split [0, Pls) into <=512-wide pieces (one PSUM bank each)."""
    out = []
    o = 0
    while o < Pls:
        w = min(512, Pls - o)
        out.append((o, w))
        o += w
    return out


def build_nc(specs):
    nc = bacc.Bacc("TRN2", target_bir_lowering=False, debug=False)

    v1t, v2t, v1e, v2e, o1T, out2, sa_d = [], [], [], [], [], [], []
    for j, sp in enumerate(specs):
        Ppl, Ppm, PCL, PCM, Pls = sp["Ppl"], sp["Ppm"], sp["PCL"], sp["PCM"], sp["Pls"]
        v1t.append(nc.dram_tensor(f"v1t{j}", [128, 2 * Ppl], FP16, kind="ExternalInput").ap())
        v2t.append(nc.dram_tensor(f"v2t{j}", [128, 2 * Ppm], FP16, kind="ExternalInput").ap())
        v1e.append(nc.dram_tensor(f"v1e{j}", [128, PCL, EW], BF16, kind="ExternalInput").ap())
        v2e.append(nc.dram_tensor(f"v2e{j}", [128, PCM, D], BF16, kind="ExternalInput").ap())
        o1T.append(nc.dram_tensor(f"o1T{j}", [128, 2 * Pls], BF16, kind="ExternalOutput").ap())
        out2.append(nc.dram_tensor(f"out2_{j}", [128, PCM, EW], BF16, kind="ExternalOutput").ap())
        sa_d.append(nc.dram_tensor(f"sa{j}", [128, PCL], F32, kind="ExternalOutput").ap())
    ident_d = nc.dram_tensor("ident", [128, 128], BF16, kind="ExternalInput").ap()
    cbias_d = nc.dram_tensor("cbias", [128, 1], F32, kind="ExternalInput").ap()

    with tile.TileContext(nc) as tc, ExitStack() as ctx:
        const_pool = ctx.enter_context(tc.tile_pool(name="const", bufs=1))
        in_pool = ctx.enter_context(tc.tile_pool(name="inp", bufs=2))
        e_pool = ctx.enter_context(tc.tile_pool(name="epool", bufs=1))
        ebpool = ctx.enter_context(tc.tile_pool(name="ebpool", bufs=2))
        out_pool = ctx.enter_context(tc.tile_pool(name="outp", bufs=2))
        sa_pool = ctx.enter_context(tc.tile_pool(name="sa", bufs=2))
        psb_pool = ctx.enter_context(tc.tile_pool(name="psb", bufs=2, space="PSUM"))
        # psT (transpose, bf16 <=1280B) and psA (attend-A accum, f32 <=2048B)
        # share one pool: their live ranges are phase-disjoint within a slot.
        pst_pool = ctx.enter_context(tc.tile_pool(name="pst", bufs=2, space="PSUM"))
        pso_pool = ctx.enter_context(tc.tile_pool(name="pso", bufs=2, space="PSUM"))

        ident = const_pool.tile([128, 128], BF16)
        nc.sync.dma_start(ident[:], ident_d)

        # The Pms trim leaves columns [Pms, Ppm) of each Eb row-chunk
        # unwritten; the transposes feed those cells into the attend-A
        # contraction where a NaN bit-pattern from uninitialized SBUF would
        # poison valid rows (0 * NaN = NaN). Zero the full region once:
        # every later write to it is finite exp output.
        eb_max = max(sp["PCL"] * sp["Ppm"] for sp in specs)
        for _ in range(2):
            eb0 = ebpool.tile([128, eb_max], BF16, tag="Eb")
            nc.gpsimd.memset(eb0[:], 0.0)
        cbias = const_pool.tile([128, 1], F32)
        nc.sync.dma_start(cbias[:], cbias_d)

        # Dummy exp so the ACT engine's Exp table load (~1.3us) happens
        # during the initial input-DMA wait instead of right before the
        # first real exp on the critical path.
        scratch = const_pool.tile([128, 2], F32)
        nc.scalar.activation(scratch[:], ident[:, 0:2], AF.Exp,
                             bias=cbias[:], scale=1.0)

        # PE warmup: dummy transposes while the first slot's input DMAs
        # stream in, so the HAM clock-gate is at 2.4 GHz when real matmuls
        # start.
        warm = pst_pool.tile([128, 512], BF16, tag="psT")
        for w in range(48):
            nc.tensor.transpose(warm[:, (w % 4) * 128:(w % 4 + 1) * 128], ident[:], ident[:])

        # attend-A of slot j-1 is interleaved into slot j's sim phase so sim
        # matmuls absorb the psA-recycle latency between attend-A units.
        prev = None  # (Ea_sb, v2e_sb, out1T_sb, spec, j-1)

        def attend_a_unit(u, st):
            """One (d-half, l-piece) of attend-A for slot jp: accumulate
            over all m-chunks into a 1-bank PSUM tile, then plain-copy out
            (unnormalized; host divides by S_a)."""
            dh, (lp0, lpw) = u
            Ea_p, v2e_p, out1T_p, spp, jp = st
            Pls_p = spp["Pls"]
            psA = pst_pool.tile([128, lpw], F32, tag="psT")
            for mc in range(spp["PCM"]):
                nc.tensor.matmul(
                    psA[:],
                    v2e_p[:, mc * D + dh * 128: mc * D + (dh + 1) * 128],
                    Ea_p[:, mc * Pls_p + lp0: mc * Pls_p + lp0 + lpw],
                    start=(mc == 0),
                    stop=(mc == spp["PCM"] - 1),
                )
            dst0 = dh * Pls_p + lp0
            nc.vector.tensor_copy(out1T_p[:, dst0:dst0 + lpw], psA[:])
            nc.sync.dma_start(o1T[jp][:, dst0:dst0 + lpw], out1T_p[:, dst0:dst0 + lpw])

        for j, sp in enumerate(specs):
            PCL, PCM = sp["PCL"], sp["PCM"]
            Ppl, Ppm, Pms, Pls, wl = sp["Ppl"], sp["Ppm"], sp["Pms"], sp["Pls"], sp["wl"]

            v1t_sb = in_pool.tile([128, 2 * Ppl], FP16, tag="v1t")
            v2t_sb = in_pool.tile([128, 2 * Ppm], FP16, tag="v2t")
            v1e_sb = in_pool.tile([128, PCL * EW], BF16, tag="v1e")
            v2e_sb = in_pool.tile([128, PCM * D], BF16, tag="v2e")
            if j == 0:
                nc.sync.dma_start(v1t_sb[:, 0:Ppl], v1t[j][:, 0:Ppl])
                nc.sync.dma_start(v2t_sb[:, 0:Ppm], v2t[j][:, 0:Ppm])
                nc.sync.dma_start(v1t_sb[:, Ppl:2 * Ppl], v1t[j][:, Ppl:2 * Ppl])
                nc.sync.dma_start(v2t_sb[:, Ppm:2 * Ppm], v2t[j][:, Ppm:2 * Ppm])
            else:
                nc.sync.dma_start(v1t_sb[:], v1t[j])
                nc.sync.dma_start(v2t_sb[:], v2t[j])
            nc.sync.dma_start(v1e_sb[:, 0:PCL * EW], v1e[j].rearrange("p c j -> p (c j)"))
            nc.sync.dma_start(v2e_sb[:, 0:PCM * D], v2e[j].rearrange("p c j -> p (c j)"))

            Eb_sb = ebpool.tile([128, PCL * Ppm], BF16, tag="Eb")
            Ea_sb = e_pool.tile([128, PCM * Pls], BF16, tag="Ea")
            Sa_sb = sa_pool.tile([128, PCL], F32, tag="Sa")
            out1T_sb = out_pool.tile([128, 2 * Pls], BF16, tag="o1")
            out2_sb = out_pool.tile([128, PCM * EW], BF16, tag="o2")

            # ---- Phase 1: sim in [l, m] layout + fused shift/exp -> Eb
            # (accum_out gives S_a row sums for free; attend-A units of
            # slot j-1 interleaved between sim groups). Columns beyond Pms
            # of each Eb row-chunk are stale: they only feed discarded
            # output rows (m >= n2) or get multiplied by zero pad rows.
            units = [] if prev is None else [
                (dh, lp) for dh in (0, 1) for lp in _lp_pieces(prev[3]["Pls"])
            ]
            n_inter = min(len(units), max(PCL - 1, 0))
            ui = 0
            for lc in range(PCL):
                psB = psb_pool.tile([128, Ppm], F32, tag="psB")
                for k in range(2):
                    for mh in range(0, Pms, 512):
                        mw = min(512, Pms - mh)
                        nc.tensor.matmul(
                            psB[:, mh:mh + mw],
                            v1t_sb[:, k * Ppl + lc * 128: k * Ppl + (lc + 1) * 128],
                            v2t_sb[:, k * Ppm + mh: k * Ppm + mh + mw],
                            start=(k == 0),
                            stop=(k == 1),
                        )
                nc.scalar.activation(
                    Eb_sb[:, lc * Ppm: lc * Ppm + Pms],
                    psB[:, 0:Pms],
                    AF.Exp,
                    bias=cbias[:],
                    scale=1.0,
                    accum_out=Sa_sb[:, lc:lc + 1],
                )
                if prev is not None and 1 <= lc and ui < n_inter:
                    attend_a_unit(units[ui], prev)
                    ui += 1

            nc.sync.dma_start(sa_d[j][:], Sa_sb[:, 0:PCL])
            while ui < len(units):
                attend_a_unit(units[ui], prev)
                ui += 1

            # ---- Phase 2+3 interleaved: attend-B groups between transpose
            # groups so the PSUM->SBUF copies (DVE) overlap attend-B matmul
            # time on the PE instead of gating it. attend-B output (with the
            # S_b ones-column) is drained unnormalized by ACT copies.
            psT_cur = [None]
            g0 = (PCL + 1) // 2  # transposes emitted before the attend-B group

            def transp_half(mc, g):
                if g == 0:
                    psT_new = pst_pool.tile([128, Ppl], BF16, tag="psT")
                    psT_cur[0] = psT_new
                psT = psT_cur[0]
                lcs = range(g0) if g == 0 else range(g0, PCL)
                for lc in lcs:
                    w = wl if lc == PCL - 1 else 128
                    nc.tensor.transpose(
                        psT[:, lc * 128: lc * 128 + w],
                        Eb_sb[:, lc * Ppm + mc * 128: lc * Ppm + (mc + 1) * 128],
                        ident[:, 0:w],
                    )
                if g == 0:
                    return
                nc.vector.tensor_copy(Ea_sb[:, mc * Pls:(mc + 1) * Pls], psT[:, 0:Pls])

            for c in range(PCM):
                transp_half(c, 0)
                psO2 = pso_pool.tile([128, EW], F32, tag="psO")
                for k in range(PCL):
                    nc.tensor.matmul(
                        psO2[:],
                        Eb_sb[:, k * Ppm + c * 128: k * Ppm + (c + 1) * 128],
                        v1e_sb[:, k * EW:(k + 1) * EW],
                        start=(k == 0),
                        stop=(k == PCL - 1),
                    )
                nc.vector.tensor_copy(out2_sb[:, c * EW:(c + 1) * EW], psO2[:])
                transp_half(c, 1)
            nc.sync.dma_start(out2[j].rearrange("p c j -> p (c j)"), out2_sb[:, 0:PCM * EW])

            prev = (Ea_sb, v2e_sb, out1T_sb, sp, j)

        for u in [(dh, lp) for dh in (0, 1) for lp in _lp_pieces(prev[3]["Pls"])]:
            attend_a_unit(u, prev)

    nc.compile()
    return nc


def _prep_slot_inputs(sp, v1b, n1b, v2b, n2b):
    """One batch -> the slot's input tensors. v1b/v2b [L, D] f32 full rows;
    n1b/n2b keep counts after compaction (rows [0:n) valid, rest zero)."""
    f32 = np.float32
    bf = ml_dtypes.bfloat16
    Ppl, Ppm, PCL, PCM = sp["Ppl"], sp["Ppm"], sp["PCL"], sp["PCM"]
    v1c = np.zeros((Ppl, D), f32)
    v1c[:n1b] = v1b[:n1b]
    v2c = np.zeros((Ppm, D), f32)
    v2c[:n2b] = v2b[:n2b]
    ones1 = np.zeros((Ppl, 1), f32)
    ones1[:n1b] = 1.0
    zeros = np.zeros((Ppl, 1), f32)
    v1e = np.concatenate([v1c, ones1, zeros], axis=1).reshape(PCL, 128, EW)
    v2e = v2c.reshape(PCM, 128, D)
    return {
        "v1t": np.ascontiguousarray(
            v1c.T.reshape(2, 128, Ppl).transpose(1, 0, 2).reshape(128, 2 * Ppl)
        ).astype(np.float16),
        "v2t": np.ascontiguousarray(
            v2c.T.reshape(2, 128, Ppm).transpose(1, 0, 2).reshape(128, 2 * Ppm)
        ).astype(np.float16),
        "v1e": np.ascontiguousarray(v1e.transpose(1, 0, 2)).astype(bf),
        "v2e": np.ascontiguousarray(v2e.transpose(1, 0, 2)).astype(bf),
    }


def run_on_hw(v1, v1_mask, v2, v2_mask, trace=False, nc=None, plan=None):
    i1s = [np.flatnonzero(~v1_mask[b]) for b in range(B)]
    i2s = [np.flatnonzero(~v2_mask[b]) for b in range(B)]
    n1 = np.array([len(i) for i in i1s])
    n2 = np.array([len(i) for i in i2s])
    if plan is None:
        plan = plan_slots(n1, n2)
    groups, specs = plan
    if nc is None:
        nc = build_nc(specs)
    in_maps = [{"ident": np.eye(128, dtype=ml_dtypes.bfloat16),
                "cbias": np.full((128, 1), -C_SHIFT, np.float32)} for _ in range(NCORES)]
    for j, (g, sp) in enumerate(zip(groups, specs)):
        for i, gb in enumerate(g):
            v1c = v1[gb][i1s[gb]]
            v2c = v2[gb][i2s[gb]]
            m = _prep_slot_inputs(sp, v1c, len(i1s[gb]), v2c, len(i2s[gb]))
            for nm, arr in m.items():
                in_maps[i][f"{nm}{j}"] = arr
    res = run_bass_kernel_spmd(nc, in_maps, core_ids=list(range(NCORES)), trace=trace)
    a1 = np.zeros((B, L, D), np.float32)
    a2 = np.zeros((B, L, D), np.float32)
    for j, (g, sp) in enumerate(zip(groups, specs)):
        Pls, PCM = sp["Pls"], sp["PCM"]
        for i, gb in enumerate(g):
            r = res.results[i]
            n1b, n2b = len(i1s[gb]), len(i2s[gb])
            # o1T [128(dpart), 2*Pls] -> [Pls, 256] unnormalized, / S_a
            u1 = r[f"o1T{j}"].reshape(128, 2, Pls).transpose(2, 1, 0).reshape(Pls, 2 * 128)
            sa = r[f"sa{j}"].T.reshape(-1)  # [Ppl]
            a1[gb, i1s[gb]] = (u1[:n1b].astype(np.float32)
                               / np.maximum(sa[:n1b, None], 1e-30))
            # out2 [128, PCM, EW] -> [Ppm, EW]; col D = S_b
            u2 = r[f"out2_{j}"].transpose(1, 0, 2).reshape(PCM * 128, EW)[:n2b]
            u2 = u2.astype(np.float32)
            a2[gb, i2s[gb]] = u2[:, 0:D] / np.maximum(u2[:, D:D + 1], 1e-30)
    return (a1, a2), res


def kernel(v1, v1_mask, v2, v2_mask):
    v1 = np.asarray(v1, np.float32)
    v2 = np.asarray(v2, np.float32)
    v1_mask = np.asarray(v1_mask)
    v2_mask = np.asarray(v2_mask)
    (a1, a2), _ = run_on_hw(v1, v1_mask, v2, v2_mask, trace=False)
    return a1, a2


# revision 9
# speedup vs baseline: 1.0574x; 1.0340x over previous
"""Bidirectional attention kernel for Trainium2 (8 NeuronCores, data-parallel
over batch) with host-side mask compaction and per-slot adaptive shapes.

~50% of rows on each side are padding (mask True). Masked rows contribute
exp(MASK_FILL - max) ~ 0 to the softmax sums, and their output rows are
zeroed. So we gather the keep rows on the host, and run dense bidirectional
attention on compacted [n1, n2] similarity slabs. Zero-padded rows
self-mask: v=0 -> sim=0 -> exp(0-88) ~ 6e-39 ~ 0 in bf16, and their
ones-column entry is 0 so they don't touch denominators.

The 64 batches are grouped into 8 program slots of 8 (one batch per core
per slot, SPMD) so that batches with the same chunk shape share a slot:
slot j is compiled for (PCL_j, PCM_j) 128-chunks = the max keep counts in
its group. Groups are formed per exact chunk-class to minimize the summed
chunk products (PE work is ~ PCL*PCM).

Math (per slot, compacted):
  sim[l, m] = v1c[l] . v2c[m]                    (fp16 matmuls, [l,m] layout)
  E[l, m]   = exp(sim - C)  (ACT, accum_out -> S_a[l] = sum_m E[l,m])
  Ea[m, l]  = transpose(E)  (PE transpose + DVE copy psum->sbuf)
  u1T[d, l] = sum_m v2e[m, d] Ea[m, l]   (v2-stationary matmuls, PSUM accum
              over m-chunks; drained by plain DVE copies, UNNORMALIZED)
  u2[m, :]  = sum_l E[l, m] v1e[l, :]    (ones col in v1e -> S_b at col D;
              drained UNNORMALIZED by ACT copies)
  host: attended_v1[l] = u1T[:, l] / S_a[l]   (S_a DMA'd out per slot)
        attended_v2[m] = u2[m, 0:D] / u2[m, D]

C is a fixed softmax shift (exp(x-C) instead of exp(x-max)): mathematically
identical softmax, safe because |sim| <~ 91 << 176 and underflow terms are
negligible relative to row sums.
"""
import sys
import types

import ml_dtypes
import numpy as np
from contextlib import ExitStack


def _install_axon_hooks_shim():
    """The image's antenv package lacks the optional axon_hooks module that
    the axon boot shim uses to register the NTFF profiling hook (it degrades
    silently without it). Provide it and redo the registration the boot shim
    skipped, so trace=True works."""
    if "antenv.axon_hooks" not in sys.modules:
        mod = types.ModuleType("antenv.axon_hooks")
        mod._hook = None

        def set_axon_ntff_profile_hook(hook):
            mod._hook = hook

        def get_axon_ntff_profile_hook():
            return mod._hook

        mod.set_axon_ntff_profile_hook = set_axon_ntff_profile_hook
        mod.get_axon_ntff_profile_hook = get_axon_ntff_profile_hook
        sys.modules["antenv.axon_hooks"] = mod
        try:
            import antenv

            antenv.axon_hooks = mod
        except ImportError:
            pass
    mod = sys.modules["antenv.axon_hooks"]
    if getattr(mod, "_hook", None) is None:
        try:
            from trn_agent_boot.trn_boot import _ntff_profile_via_ctypes

            mod._hook = _ntff_profile_via_ctypes("/opt/axon/libaxon_pjrt.so")
        except Exception:
            pass


_install_axon_hooks_shim()

import concourse.bacc as bacc
import concourse.mybir as mybir
import concourse.tile as tile
from concourse.bass_utils import run_bass_kernel_spmd

F32 = mybir.dt.float32
BF16 = mybir.dt.bfloat16
FP16 = mybir.dt.float16
AF = mybir.ActivationFunctionType
ALU = mybir.AluOpType

B, L, D = 64, 1024, 256
NCORES = 8
BPC = B // NCORES          # batches per core == number of program slots
C_SHIFT = np.float32(88.0)
EW = D + 2  # v1e free width: col 256 = ones (S_b denominator), col 257 = 0
# pad (even free-dim count for the ISA)


def _even_ceil(x):
    return int(x + (x & 1))


def plan_slots(n1, n2):
    """Group the 64 batches into BPC slots of NCORES batches with matching
    chunk shapes. Returns (groups, specs): groups[j] = list of NCORES batch
    ids (core i takes groups[j][i]); specs[j] = dict of per-slot shapes."""
    cl = np.maximum(1, -(-n1 // 128)).astype(int)
    cm = np.maximum(1, -(-n2 // 128)).astype(int)
    classes = {}
    for b in range(B):
        classes.setdefault((int(cl[b]), int(cm[b])), []).append(b)
    groups = []
    leftovers = []
    for key in sorted(classes, key=lambda k: (-k[0] * k[1], -k[0])):
        lst = classes[key]
        while len(lst) >= NCORES:
            groups.append(lst[:NCORES])
            lst = lst[NCORES:]
        leftovers.extend(lst)
    leftovers.sort(key=lambda b: (-int(cl[b] * cm[b]), -int(cl[b])))
    while leftovers:
        groups.append(leftovers[:NCORES])
        leftovers = leftovers[NCORES:]
    assert len(groups) == BPC
    # order: smallest slot first (fast pipeline fill), second-smallest last
    # (short attend-A tail), rest in the middle
    groups.sort(key=lambda g: max(cl[b] * cm[b] for b in g))
    groups = [groups[0]] + groups[2:] + [groups[1]]
    specs = []
    for g in groups:
        PCL = int(max(cl[b] for b in g))
        PCM = int(max(cm[b] for b in g))
        mx1 = int(max(n1[b] for b in g))
        mx2 = int(max(n2[b] for b in g))
        specs.append(dict(
            PCL=PCL, PCM=PCM,
            Ppl=PCL * 128, Ppm=PCM * 128,
            Pms=min(PCM * 128, _even_ceil(mx2)),   # sim/exp m free width
            Pls=min(PCL * 128, _even_ceil(mx1)),   # attend-A l free width
            # transpose out width for the last l-chunk
            wl=min(128, _even_ceil(mx1 - (PCL - 1) * 128)),
        ))
    return groups, specs


def _lp_pieces(Pls):
    """Split [0, Pls) into <=512-wide pieces (one PSUM bank each)."""
    out = []
    o = 0
    while o < Pls:
        w = min(512, Pls - o)
        out.append((o, w))
        o += w
    return out


def build_nc(specs):
    nc = bacc.Bacc("TRN2", target_bir_lowering=False, debug=False)

    v1t, v2t, v1e, v2e, o1T, out2, sa_d = [], [], [], [], [], [], []
    for j, sp in enumerate(specs):
        Ppl, Ppm, PCL, PCM, Pls = sp["Ppl"], sp["Ppm"], sp["PCL"], sp["PCM"], sp["Pls"]
        v1t.append(nc.dram_tensor(f"v1t{j}", [128, 2 * Ppl], FP16, kind="ExternalInput").ap())
        v2t.append(nc.dram_tensor(f"v2t{j}", [128, 2 * Ppm], FP16, kind="ExternalInput").ap())
        v1e.append(nc.dram_tensor(f"v1e{j}", [128, PCL, EW], BF16, kind="ExternalInput").ap())
        v2e.append(nc.dram_tensor(f"v2e{j}", [128, PCM, D], BF16, kind="ExternalInput").ap())
        o1T.append(nc.dram_tensor(f"o1T{j}", [128, 2 * Pls], BF16, kind="ExternalOutput").ap())
        out2.append(nc.dram_tensor(f"out2_{j}", [128, PCM, EW], BF16, kind="ExternalOutput").ap())
        sa_d.append(nc.dram_tensor(f"sa{j}", [128, PCL], F32, kind="ExternalOutput").ap())
    ident_d = nc.dram_tensor("ident", [128, 128], BF16, kind="ExternalInput").ap()
    cbias_d = nc.dram_tensor("cbias", [128, 1], F32, kind="ExternalInput").ap()

    with tile.TileContext(nc) as tc, ExitStack() as ctx:
        const_pool = ctx.enter_context(tc.tile_pool(name="const", bufs=1))
        # bufs=3 so slot j+1's input DMAs can be issued at the start of slot
        # j's body without the Sync queue blocking on the j-1 tile recycle.
        in_pool = ctx.enter_context(tc.tile_pool(name="inp", bufs=3))
        e_pool = ctx.enter_context(tc.tile_pool(name="epool", bufs=1))
        ebpool = ctx.enter_context(tc.tile_pool(name="ebpool", bufs=2))
        out_pool = ctx.enter_context(tc.tile_pool(name="outp", bufs=2))
        sa_pool = ctx.enter_context(tc.tile_pool(name="sa", bufs=2))
        psb_pool = ctx.enter_context(tc.tile_pool(name="psb", bufs=2, space="PSUM"))
        # psT (transpose, bf16 <=1280B) and psA (attend-A accum, f32 <=2048B)
        # share one pool: their live ranges are phase-disjoint within a slot.
        pst_pool = ctx.enter_context(tc.tile_pool(name="pst", bufs=2, space="PSUM"))
        pso_pool = ctx.enter_context(tc.tile_pool(name="pso", bufs=2, space="PSUM"))

        ident = const_pool.tile([128, 128], BF16)
        nc.sync.dma_start(ident[:], ident_d)

        # The Pms trim leaves columns [Pms, Ppm) of each Eb row-chunk
        # unwritten; the transposes feed those cells into the attend-A
        # contraction where a NaN bit-pattern from uninitialized SBUF would
        # poison valid rows (0 * NaN = NaN). Zero the full region once:
        # every later write to it is finite exp output.
        eb_max = max(sp["PCL"] * sp["Ppm"] for sp in specs)
        for _ in range(2):
            eb0 = ebpool.tile([128, eb_max], BF16, tag="Eb")
            nc.gpsimd.memset(eb0[:], 0.0)
        cbias = const_pool.tile([128, 1], F32)
        nc.sync.dma_start(cbias[:], cbias_d)

        # Dummy exp so the ACT engine's Exp table load (~1.3us) happens
        # during the initial input-DMA wait instead of right before the
        # first real exp on the critical path.
        scratch = const_pool.tile([128, 2], F32)
        nc.scalar.activation(scratch[:], ident[:, 0:2], AF.Exp,
                             bias=cbias[:], scale=1.0)

        # PE warmup: dummy transposes while the first slot's input DMAs
        # stream in, so the HAM clock-gate is at 2.4 GHz when real matmuls
        # start.
        warm = pst_pool.tile([128, 512], BF16, tag="psT")
        for w in range(48):
            nc.tensor.transpose(warm[:, (w % 4) * 128:(w % 4 + 1) * 128], ident[:], ident[:])

        # attend-A of slot j-1 is interleaved into slot j's sim phase so sim
        # matmuls absorb the psA-recycle latency between attend-A units.
        prev = None  # (Ea_sb, v2e_sb, out1T_sb, spec, j-1)

        def attend_a_unit(u, st):
            """One (d-half, l-piece) of attend-A for slot jp: accumulate
            over all m-chunks into a 1-bank PSUM tile, then plain-copy out
            (unnormalized; host divides by S_a)."""
            dh, (lp0, lpw) = u
            Ea_p, v2e_p, out1T_p, spp, jp = st
            Pls_p = spp["Pls"]
            psA = pst_pool.tile([128, lpw], F32, tag="psT")
            for mc in range(spp["PCM"]):
                nc.tensor.matmul(
                    psA[:],
                    v2e_p[:, mc * D + dh * 128: mc * D + (dh + 1) * 128],
                    Ea_p[:, mc * Pls_p + lp0: mc * Pls_p + lp0 + lpw],
                    start=(mc == 0),
                    stop=(mc == spp["PCM"] - 1),
                )
            dst0 = dh * Pls_p + lp0
            nc.vector.tensor_copy(out1T_p[:, dst0:dst0 + lpw], psA[:])
            # output DMA starts go on the (idle) GpSimd queue so the Sync
            # queue never head-of-line blocks input prefetch behind them
            nc.gpsimd.dma_start(o1T[jp][:, dst0:dst0 + lpw], out1T_p[:, dst0:dst0 + lpw])

        def issue_inputs(j):
            sp = specs[j]
            Ppl, Ppm, PCL, PCM = sp["Ppl"], sp["Ppm"], sp["PCL"], sp["PCM"]
            v1t_sb = in_pool.tile([128, 2 * Ppl], FP16, tag="v1t")
            v2t_sb = in_pool.tile([128, 2 * Ppm], FP16, tag="v2t")
            v1e_sb = in_pool.tile([128, PCL * EW], BF16, tag="v1e")
            v2e_sb = in_pool.tile([128, PCM * D], BF16, tag="v2e")
            # halves ordered so the k=0 operands (first sim MMs) land first
            nc.sync.dma_start(v1t_sb[:, 0:Ppl], v1t[j][:, 0:Ppl])
            nc.sync.dma_start(v2t_sb[:, 0:Ppm], v2t[j][:, 0:Ppm])
            nc.sync.dma_start(v1t_sb[:, Ppl:2 * Ppl], v1t[j][:, Ppl:2 * Ppl])
            nc.sync.dma_start(v2t_sb[:, Ppm:2 * Ppm], v2t[j][:, Ppm:2 * Ppm])
            nc.sync.dma_start(v1e_sb[:, 0:PCL * EW], v1e[j].rearrange("p c j -> p (c j)"))
            nc.sync.dma_start(v2e_sb[:, 0:PCM * D], v2e[j].rearrange("p c j -> p (c j)"))
            return v1t_sb, v2t_sb, v1e_sb, v2e_sb

        in_tiles = issue_inputs(0)
        for j, sp in enumerate(specs):
            PCL, PCM = sp["PCL"], sp["PCM"]
            Ppl, Ppm, Pms, Pls, wl = sp["Ppl"], sp["Ppm"], sp["Pms"], sp["Pls"], sp["wl"]

            v1t_sb, v2t_sb, v1e_sb, v2e_sb = in_tiles
            if j + 1 < len(specs):
                in_tiles = issue_inputs(j + 1)

            Eb_sb = ebpool.tile([128, PCL * Ppm], BF16, tag="Eb")
            Ea_sb = e_pool.tile([128, PCM * Pls], BF16, tag="Ea")
            Sa_sb = sa_pool.tile([128, PCL], F32, tag="Sa")
            out1T_sb = out_pool.tile([128, 2 * Pls], BF16, tag="o1")
            out2_sb = out_pool.tile([128, PCM * EW], BF16, tag="o2")

            # ---- Phase 1: sim in [l, m] layout + fused shift/exp -> Eb
            # (accum_out gives S_a row sums for free; attend-A units of
            # slot j-1 interleaved between sim groups). Columns beyond Pms
            # of each Eb row-chunk are stale: they only feed discarded
            # output rows (m >= n2) or get multiplied by zero pad rows.
            units = [] if prev is None else [
                (dh, lp) for dh in (0, 1) for lp in _lp_pieces(prev[3]["Pls"])
            ]
            n_inter = min(len(units), max(PCL - 1, 0))
            ui = 0
            for lc in range(PCL):
                psB = psb_pool.tile([128, Ppm], F32, tag="psB")
                for k in range(2):
                    for mh in range(0, Pms, 512):
                        mw = min(512, Pms - mh)
                        nc.tensor.matmul(
                            psB[:, mh:mh + mw],
                            v1t_sb[:, k * Ppl + lc * 128: k * Ppl + (lc + 1) * 128],
                            v2t_sb[:, k * Ppm + mh: k * Ppm + mh + mw],
                            start=(k == 0),
                            stop=(k == 1),
                        )
                nc.scalar.activation(
                    Eb_sb[:, lc * Ppm: lc * Ppm + Pms],
                    psB[:, 0:Pms],
                    AF.Exp,
                    bias=cbias[:],
                    scale=1.0,
                    accum_out=Sa_sb[:, lc:lc + 1],
                )
                if prev is not None and 1 <= lc and ui < n_inter:
                    attend_a_unit(units[ui], prev)
                    ui += 1

            nc.gpsimd.dma_start(sa_d[j][:], Sa_sb[:, 0:PCL])
            while ui < len(units):
                attend_a_unit(units[ui], prev)
                ui += 1

            # ---- Phase 2+3 interleaved: attend-B groups between transpose
            # groups so the PSUM->SBUF copies (DVE) overlap attend-B matmul
            # time on the PE instead of gating it. attend-B output (with the
            # S_b ones-column) is drained unnormalized by ACT copies.
            psT_cur = [None]
            g0 = (PCL + 1) // 2  # transposes emitted before the attend-B group

            def transp_half(mc, g):
                if g == 0:
                    psT_new = pst_pool.tile([128, Ppl], BF16, tag="psT")
                    psT_cur[0] = psT_new
                psT = psT_cur[0]
                lcs = range(g0) if g == 0 else range(g0, PCL)
                for lc in lcs:
                    w = wl if lc == PCL - 1 else 128
                    nc.tensor.transpose(
                        psT[:, lc * 128: lc * 128 + w],
                        Eb_sb[:, lc * Ppm + mc * 128: lc * Ppm + (mc + 1) * 128],
                        ident[:, 0:w],
                    )
                if g == 0:
                    return
                nc.vector.tensor_copy(Ea_sb[:, mc * Pls:(mc + 1) * Pls], psT[:, 0:Pls])

            for c in range(PCM):
                transp_half(c, 0)
                psO2 = pso_pool.tile([128, EW], F32, tag="psO")
                for k in range(PCL):
                    nc.tensor.matmul(
                        psO2[:],
                        Eb_sb[:, k * Ppm + c * 128: k * Ppm + (c + 1) * 128],
                        v1e_sb[:, k * EW:(k + 1) * EW],
                        start=(k == 0),
                        stop=(k == PCL - 1),
                    )
                nc.vector.tensor_copy(out2_sb[:, c * EW:(c + 1) * EW], psO2[:])
                transp_half(c, 1)
            nc.gpsimd.dma_start(out2[j].rearrange("p c j -> p (c j)"), out2_sb[:, 0:PCM * EW])

            prev = (Ea_sb, v2e_sb, out1T_sb, sp, j)

        for u in [(dh, lp) for dh in (0, 1) for lp in _lp_pieces(prev[3]["Pls"])]:
            attend_a_unit(u, prev)

    nc.compile()
    return nc


def _prep_slot_inputs(sp, v1b, n1b, v2b, n2b):
    """One batch -> the slot's input tensors. v1b/v2b [L, D] f32 full rows;
    n1b/n2b keep counts after compaction (rows [0:n) valid, rest zero)."""
    f32 = np.float32
    bf = ml_dtypes.bfloat16
    Ppl, Ppm, PCL, PCM = sp["Ppl"], sp["Ppm"], sp["PCL"], sp["PCM"]
    v1c = np.zeros((Ppl, D), f32)
    v1c[:n1b] = v1b[:n1b]
    v2c = np.zeros((Ppm, D), f32)
    v2c[:n2b] = v2b[:n2b]
    ones1 = np.zeros((Ppl, 1), f32)
    ones1[:n1b] = 1.0
    zeros = np.zeros((Ppl, 1), f32)
    v1e = np.concatenate([v1c, ones1, zeros], axis=1).reshape(PCL, 128, EW)
    v2e = v2c.reshape(PCM, 128, D)
    return {
        "v1t": np.ascontiguousarray(
            v1c.T.reshape(2, 128, Ppl).transpose(1, 0, 2).reshape(128, 2 * Ppl)
        ).astype(np.float16),
        "v2t": np.ascontiguousarray(
            v2c.T.reshape(2, 128, Ppm).transpose(1, 0, 2).reshape(128, 2 * Ppm)
        ).astype(np.float16),
        "v1e": np.ascontiguousarray(v1e.transpose(1, 0, 2)).astype(bf),
        "v2e": np.ascontiguousarray(v2e.transpose(1, 0, 2)).astype(bf),
    }


def run_on_hw(v1, v1_mask, v2, v2_mask, trace=False, nc=None, plan=None):
    i1s = [np.flatnonzero(~v1_mask[b]) for b in range(B)]
    i2s = [np.flatnonzero(~v2_mask[b]) for b in range(B)]
    n1 = np.array([len(i) for i in i1s])
    n2 = np.array([len(i) for i in i2s])
    if plan is None:
        plan = plan_slots(n1, n2)
    groups, specs = plan
    if nc is None:
        nc = build_nc(specs)
    in_maps = [{"ident": np.eye(128, dtype=ml_dtypes.bfloat16),
                "cbias": np.full((128, 1), -C_SHIFT, np.float32)} for _ in range(NCORES)]
    for j, (g, sp) in enumerate(zip(groups, specs)):
        for i, gb in enumerate(g):
            v1c = v1[gb][i1s[gb]]
            v2c = v2[gb][i2s[gb]]
            m = _prep_slot_inputs(sp, v1c, len(i1s[gb]), v2c, len(i2s[gb]))
            for nm, arr in m.items():
                in_maps[i][f"{nm}{j}"] = arr
    res = run_bass_kernel_spmd(nc, in_maps, core_ids=list(range(NCORES)), trace=trace)
    a1 = np.zeros((B, L, D), np.float32)
    a2 = np.zeros((B, L, D), np.float32)
    for j, (g, sp) in enumerate(zip(groups, specs)):
        Pls, PCM = sp["Pls"], sp["PCM"]
        for i, gb in enumerate(g):
            r = res.results[i]
            n1b, n2b = len(i1s[gb]), len(i2s[gb])
            # o1T [128(dpart), 2*Pls] -> [Pls, 256] unnormalized, / S_a
            u1 = r[f"o1T{j}"].reshape(128, 2, Pls).transpose(2, 1, 0).reshape(Pls, 2 * 128)
            sa = r[f"sa{j}"].T.reshape(-1)  # [Ppl]
            a1[gb, i1s[gb]] = (u1[:n1b].astype(np.float32)
                               / np.maximum(sa[:n1b, None], 1e-30))
            # out2 [128, PCM, EW] -> [Ppm, EW]; col D = S_b
            u2 = r[f"out2_{j}"].transpose(1, 0, 2).reshape(PCM * 128, EW)[:n2b]
            u2 = u2.astype(np.float32)
            a2[gb, i2s[gb]] = u2[:, 0:D] / np.maximum(u2[:, D:D + 1], 1e-30)
    return (a1, a2), res


def kernel(v1, v1_mask, v2, v2_mask):
    v1 = np.asarray(v1, np.float32)
    v2 = np.asarray(v2, np.float32)
    v1_mask = np.asarray(v1_mask)
    v2_mask = np.asarray(v2_mask)
    (a1, a2), _ = run_on_hw(v1, v1_mask, v2, v2_mask, trace=False)
    return a1, a2


# revision 16
# speedup vs baseline: 1.0927x; 1.0334x over previous
"""Bidirectional attention kernel for Trainium2 (8 NeuronCores, data-parallel
over batch) with host-side mask compaction and per-slot adaptive shapes.

~50% of rows on each side are padding (mask True). Masked rows contribute
exp(MASK_FILL - max) ~ 0 to the softmax sums, and their output rows are
zeroed. So we gather the keep rows on the host, and run dense bidirectional
attention on compacted [n1, n2] similarity slabs. Zero-padded rows
self-mask: v=0 -> sim=0 -> exp(0-88) ~ 6e-39 ~ 0 in bf16, and their
ones-column entry is 0 so they don't touch denominators.

The 64 batches are grouped into 8 program slots of 8 (one batch per core
per slot, SPMD) so that batches with the same chunk shape share a slot:
slot j is compiled for (PCL_j, PCM_j) 128-chunks = the max keep counts in
its group. Groups are formed per exact chunk-class to minimize the summed
chunk products (PE work is ~ PCL*PCM).

Math (per slot, compacted):
  sim[l, m] = v1c[l] . v2c[m]                    (fp16 matmuls, [l,m] layout)
  E[l, m]   = exp(sim - C)  (ACT, accum_out -> S_a[l] = sum_m E[l,m])
  Ea[m, l]  = transpose(E)  (PE transpose + DVE copy psum->sbuf)
  u1T[d, l] = sum_m v2e[m, d] Ea[m, l]   (v2-stationary matmuls, PSUM accum
              over m-chunks; drained by plain DVE copies, UNNORMALIZED)
  u2[m, :]  = sum_l E[l, m] v1e[l, :]    (ones col in v1e -> S_b at col D;
              drained UNNORMALIZED by ACT copies)
  host: attended_v1[l] = u1T[:, l] / S_a[l]   (S_a DMA'd out per slot)
        attended_v2[m] = u2[m, 0:D] / u2[m, D]

C is a fixed softmax shift (exp(x-C) instead of exp(x-max)): mathematically
identical softmax, safe because |sim| <~ 91 << 176 and underflow terms are
negligible relative to row sums.
"""
import sys
import types

import ml_dtypes
import numpy as np
from contextlib import ExitStack


def _install_axon_hooks_shim():
    """The image's antenv package lacks the optional axon_hooks module that
    the axon boot shim uses to register the NTFF profiling hook (it degrades
    silently without it). Provide it and redo the registration the boot shim
    skipped, so trace=True works."""
    if "antenv.axon_hooks" not in sys.modules:
        mod = types.ModuleType("antenv.axon_hooks")
        mod._hook = None

        def set_axon_ntff_profile_hook(hook):
            mod._hook = hook

        def get_axon_ntff_profile_hook():
            return mod._hook

        mod.set_axon_ntff_profile_hook = set_axon_ntff_profile_hook
        mod.get_axon_ntff_profile_hook = get_axon_ntff_profile_hook
        sys.modules["antenv.axon_hooks"] = mod
        try:
            import antenv

            antenv.axon_hooks = mod
        except ImportError:
            pass
    mod = sys.modules["antenv.axon_hooks"]
    if getattr(mod, "_hook", None) is None:
        try:
            from trn_agent_boot.trn_boot import _ntff_profile_via_ctypes

            mod._hook = _ntff_profile_via_ctypes("/opt/axon/libaxon_pjrt.so")
        except Exception:
            pass


_install_axon_hooks_shim()

import concourse.bacc as bacc
import concourse.mybir as mybir
import concourse.tile as tile
from concourse.bass_utils import run_bass_kernel_spmd

F32 = mybir.dt.float32
BF16 = mybir.dt.bfloat16
FP16 = mybir.dt.float16
AF = mybir.ActivationFunctionType
ALU = mybir.AluOpType

B, L, D = 64, 1024, 256
NCORES = 8
BPC = B // NCORES          # batches per core == number of program slots
C_SHIFT = np.float32(88.0)
EW = D + 2  # v1e free width: col 256 = ones (S_b denominator), col 257 = 0
# pad (even free-dim count for the ISA)


def _even_ceil(x):
    return int(x + (x & 1))


def plan_slots(n1, n2):
    """Group the 64 batches into BPC slots of NCORES batches with matching
    chunk shapes. Returns (groups, specs): groups[j] = list of NCORES batch
    ids (core i takes groups[j][i]); specs[j] = dict of per-slot shapes."""
    cl = np.maximum(1, -(-n1 // 128)).astype(int)
    cm = np.maximum(1, -(-n2 // 128)).astype(int)
    classes = {}
    for b in range(B):
        classes.setdefault((int(cl[b]), int(cm[b])), []).append(b)
    groups = []
    leftovers = []
    for key in sorted(classes, key=lambda k: (-k[0] * k[1], -k[0])):
        lst = classes[key]
        while len(lst) >= NCORES:
            groups.append(lst[:NCORES])
            lst = lst[NCORES:]
        leftovers.extend(lst)
    leftovers.sort(key=lambda b: (-int(cl[b] * cm[b]), -int(cl[b])))
    while leftovers:
        groups.append(leftovers[:NCORES])
        leftovers = leftovers[NCORES:]
    assert len(groups) == BPC
    # order: second-smallest first (fast pipeline fill), smallest last
    # (short attend-A + output-DMA tail), rest in the middle
    groups.sort(key=lambda g: max(cl[b] * cm[b] for b in g))
    groups = [groups[1]] + groups[2:] + [groups[0]]
    specs = []
    for g in groups:
        PCL = int(max(cl[b] for b in g))
        PCM = int(max(cm[b] for b in g))
        mx1 = int(max(n1[b] for b in g))
        mx2 = int(max(n2[b] for b in g))
        specs.append(dict(
            PCL=PCL, PCM=PCM,
            Ppl=PCL * 128, Ppm=PCM * 128,
            Pms=min(PCM * 128, _even_ceil(mx2)),   # sim/exp m free width
            Pls=min(PCL * 128, _even_ceil(mx1)),   # attend-A l free width
            # transpose out width for the last l-chunk
            wl=min(128, _even_ceil(mx1 - (PCL - 1) * 128)),
        ))
    return groups, specs


def _lp_pieces(Pls):
    """Split [0, Pls) into <=512-wide pieces (one PSUM bank each). Balanced
    halves when a split is needed, so no piece is so narrow that its matmuls
    fall under the ~107ns LDWEIGHTS floor."""
    if Pls <= 512:
        return [(0, Pls)]
    h = _even_ceil((Pls + 1) // 2)
    return [(0, h), (h, Pls - h)]


def build_nc(specs):
    nc = bacc.Bacc("TRN2", target_bir_lowering=False, debug=False)

    v1t, v2t, v1e, v2e, o1T, out2, sa_d = [], [], [], [], [], [], []
    for j, sp in enumerate(specs):
        Ppl, Ppm, PCL, PCM, Pls = sp["Ppl"], sp["Ppm"], sp["PCL"], sp["PCM"], sp["Pls"]
        v1t.append(nc.dram_tensor(f"v1t{j}", [128, 2 * Ppl], FP16, kind="ExternalInput").ap())
        v2t.append(nc.dram_tensor(f"v2t{j}", [128, 2 * Ppm], FP16, kind="ExternalInput").ap())
        v1e.append(nc.dram_tensor(f"v1e{j}", [128, PCL, EW], BF16, kind="ExternalInput").ap())
        v2e.append(nc.dram_tensor(f"v2e{j}", [128, PCM, D], BF16, kind="ExternalInput").ap())
        o1T.append(nc.dram_tensor(f"o1T{j}", [128, 2 * Pls], BF16, kind="ExternalOutput").ap())
        out2.append(nc.dram_tensor(f"out2_{j}", [128, PCM, EW], BF16, kind="ExternalOutput").ap())
        sa_d.append(nc.dram_tensor(f"sa{j}", [128, PCL], F32, kind="ExternalOutput").ap())
    ident_d = nc.dram_tensor("ident", [128, 128], BF16, kind="ExternalInput").ap()
    cbias_d = nc.dram_tensor("cbias", [128, 1], F32, kind="ExternalInput").ap()

    with tile.TileContext(nc) as tc, ExitStack() as ctx:
        const_pool = ctx.enter_context(tc.tile_pool(name="const", bufs=1))
        # bufs=3 so slot j+1's input DMAs can be issued at the start of slot
        # j's body without the Sync queue blocking on the j-1 tile recycle.
        in_pool = ctx.enter_context(tc.tile_pool(name="inp", bufs=3))
        e_pool = ctx.enter_context(tc.tile_pool(name="epool", bufs=1))
        ebpool = ctx.enter_context(tc.tile_pool(name="ebpool", bufs=2))
        out_pool = ctx.enter_context(tc.tile_pool(name="outp", bufs=2))
        sa_pool = ctx.enter_context(tc.tile_pool(name="sa", bufs=2))
        psb_pool = ctx.enter_context(tc.tile_pool(name="psb", bufs=2, space="PSUM"))
        # psT (transpose, bf16 <=1280B) and psA (attend-A accum, f32 <=2048B)
        # share one pool: their live ranges are phase-disjoint within a slot.
        pst_pool = ctx.enter_context(tc.tile_pool(name="pst", bufs=2, space="PSUM"))
        pso_pool = ctx.enter_context(tc.tile_pool(name="pso", bufs=2, space="PSUM"))

        ident = const_pool.tile([128, 128], BF16)
        nc.sync.dma_start(ident[:], ident_d)

        # The Pms trim leaves columns [Pms, Ppm) of each Eb row-chunk
        # unwritten; the transposes feed those cells into the attend-A
        # contraction where a NaN bit-pattern from uninitialized SBUF would
        # poison valid rows (0 * NaN = NaN). Zero the full region once:
        # every later write to it is finite exp output.
        eb_max = max(sp["PCL"] * sp["Ppm"] for sp in specs)
        for _ in range(2):
            eb0 = ebpool.tile([128, eb_max], BF16, tag="Eb")
            nc.gpsimd.memset(eb0[:], 0.0)
        cbias = const_pool.tile([128, 1], F32)
        nc.sync.dma_start(cbias[:], cbias_d)

        # Dummy exp so the ACT engine's Exp table load (~1.3us) happens
        # during the initial input-DMA wait instead of right before the
        # first real exp on the critical path.
        scratch = const_pool.tile([128, 2], F32)
        nc.scalar.activation(scratch[:], ident[:, 0:2], AF.Exp,
                             bias=cbias[:], scale=1.0)

        # PE warmup: dummy transposes while the first slot's input DMAs
        # stream in, so the HAM clock-gate is at 2.4 GHz when real matmuls
        # start.
        warm = pst_pool.tile([128, 512], BF16, tag="psT")
        for w in range(48):
            nc.tensor.transpose(warm[:, (w % 4) * 128:(w % 4 + 1) * 128], ident[:], ident[:])

        # attend-A of slot j-1 is interleaved into slot j's sim phase so sim
        # matmuls absorb the psA-recycle latency between attend-A units.
        prev = None  # (Ea_sb, v2e_sb, out1T_sb, spec, j-1)

        def attend_a_unit(u, st, dma_eng=None):
            """One (d-half, l-piece) of attend-A for slot jp: accumulate
            over all m-chunks into a 1-bank PSUM tile, then plain-copy out
            (unnormalized; host divides by S_a)."""
            dh, (lp0, lpw) = u
            Ea_p, v2e_p, out1T_p, spp, jp = st
            Pls_p = spp["Pls"]
            psA = pst_pool.tile([128, lpw], F32, tag="psT")
            for mc in range(spp["PCM"]):
                nc.tensor.matmul(
                    psA[:],
                    v2e_p[:, mc * D + dh * 128: mc * D + (dh + 1) * 128],
                    Ea_p[:, mc * Pls_p + lp0: mc * Pls_p + lp0 + lpw],
                    start=(mc == 0),
                    stop=(mc == spp["PCM"] - 1),
                )
            dst0 = dh * Pls_p + lp0
            nc.vector.tensor_copy(out1T_p[:, dst0:dst0 + lpw], psA[:])
            # output DMA starts go on the (idle) GpSimd queue so the Sync
            # queue never head-of-line blocks input prefetch behind them
            eng = dma_eng if dma_eng is not None else nc.gpsimd
            eng.dma_start(o1T[jp][:, dst0:dst0 + lpw], out1T_p[:, dst0:dst0 + lpw])

        def issue_inputs(j):
            sp = specs[j]
            Ppl, Ppm, PCL, PCM = sp["Ppl"], sp["Ppm"], sp["PCL"], sp["PCM"]
            v1t_sb = in_pool.tile([128, 2 * Ppl], FP16, tag="v1t")
            v2t_sb = in_pool.tile([128, 2 * Ppm], FP16, tag="v2t")
            v1e_sb = in_pool.tile([128, PCL * EW], BF16, tag="v1e")
            v2e_sb = in_pool.tile([128, PCM * D], BF16, tag="v2e")
            if j == 0:
                # fine-grained pieces ordered by first use, so the first sim
                # matmul's operands (v1t k0 chunk 0 + v2t k0 first half)
                # arrive ~2.5us before the full input set.
                h = min(512, Ppm)
                nc.sync.dma_start(v1t_sb[:, 0:128], v1t[j][:, 0:128])
                nc.sync.dma_start(v2t_sb[:, 0:h], v2t[j][:, 0:h])
                nc.sync.dma_start(v1t_sb[:, 128:Ppl], v1t[j][:, 128:Ppl])
                if h < Ppm:
                    nc.sync.dma_start(v2t_sb[:, h:Ppm], v2t[j][:, h:Ppm])
                nc.sync.dma_start(v1t_sb[:, Ppl:2 * Ppl], v1t[j][:, Ppl:2 * Ppl])
                nc.sync.dma_start(v2t_sb[:, Ppm:2 * Ppm], v2t[j][:, Ppm:2 * Ppm])
            else:
                # halves ordered so the k=0 operands (first sim MMs) land first
                nc.sync.dma_start(v1t_sb[:, 0:Ppl], v1t[j][:, 0:Ppl])
                nc.sync.dma_start(v2t_sb[:, 0:Ppm], v2t[j][:, 0:Ppm])
                nc.sync.dma_start(v1t_sb[:, Ppl:2 * Ppl], v1t[j][:, Ppl:2 * Ppl])
                nc.sync.dma_start(v2t_sb[:, Ppm:2 * Ppm], v2t[j][:, Ppm:2 * Ppm])
            nc.sync.dma_start(v1e_sb[:, 0:PCL * EW], v1e[j].rearrange("p c j -> p (c j)"))
            nc.sync.dma_start(v2e_sb[:, 0:PCM * D], v2e[j].rearrange("p c j -> p (c j)"))
            return v1t_sb, v2t_sb, v1e_sb, v2e_sb

        in_tiles = issue_inputs(0)
        for j, sp in enumerate(specs):
            PCL, PCM = sp["PCL"], sp["PCM"]
            Ppl, Ppm, Pms, Pls, wl = sp["Ppl"], sp["Ppm"], sp["Pms"], sp["Pls"], sp["wl"]

            v1t_sb, v2t_sb, v1e_sb, v2e_sb = in_tiles
            if j + 1 < len(specs):
                in_tiles = issue_inputs(j + 1)

            Eb_sb = ebpool.tile([128, PCL * Ppm], BF16, tag="Eb")
            Ea_sb = e_pool.tile([128, PCM * Pls], BF16, tag="Ea")
            Sa_sb = sa_pool.tile([128, PCL], F32, tag="Sa")
            out1T_sb = out_pool.tile([128, 2 * Pls], BF16, tag="o1")
            out2_sb = out_pool.tile([128, PCM * EW], BF16, tag="o2")

            # ---- Phase 1: sim in [l, m] layout + fused shift/exp -> Eb
            # (accum_out gives S_a row sums for free; attend-A units of
            # slot j-1 interleaved between sim groups). Columns beyond Pms
            # of each Eb row-chunk are stale: they only feed discarded
            # output rows (m >= n2) or get multiplied by zero pad rows.
            units = [] if prev is None else [
                (dh, lp) for dh in (0, 1) for lp in _lp_pieces(prev[3]["Pls"])
            ]
            n_inter = min(len(units), max(PCL - 1, 0))
            ui = 0
            for lc in range(PCL):
                psB = psb_pool.tile([128, Ppm], F32, tag="psB")
                for k in range(2):
                    for mh in range(0, Pms, 512):
                        mw = min(512, Pms - mh)
                        nc.tensor.matmul(
                            psB[:, mh:mh + mw],
                            v1t_sb[:, k * Ppl + lc * 128: k * Ppl + (lc + 1) * 128],
                            v2t_sb[:, k * Ppm + mh: k * Ppm + mh + mw],
                            start=(k == 0),
                            stop=(k == 1),
                        )
                nc.scalar.activation(
                    Eb_sb[:, lc * Ppm: lc * Ppm + Pms],
                    psB[:, 0:Pms],
                    AF.Exp,
                    bias=cbias[:],
                    scale=1.0,
                    accum_out=Sa_sb[:, lc:lc + 1],
                )
                if prev is not None and 1 <= lc and ui < n_inter:
                    attend_a_unit(units[ui], prev)
                    ui += 1

            nc.gpsimd.dma_start(sa_d[j][:], Sa_sb[:, 0:PCL])
            while ui < len(units):
                attend_a_unit(units[ui], prev)
                ui += 1

            # ---- Phase 2+3 interleaved: attend-B groups between transpose
            # groups so the PSUM->SBUF copies (DVE) overlap attend-B matmul
            # time on the PE instead of gating it. attend-B output (with the
            # S_b ones-column) is drained unnormalized by ACT copies.
            psT_cur = [None]
            g0 = (PCL + 1) // 2  # transposes emitted before the attend-B group

            def transp_half(mc, g):
                if g == 0:
                    psT_new = pst_pool.tile([128, Ppl], BF16, tag="psT")
                    psT_cur[0] = psT_new
                psT = psT_cur[0]
                lcs = range(g0) if g == 0 else range(g0, PCL)
                for lc in lcs:
                    w = wl if lc == PCL - 1 else 128
                    nc.tensor.transpose(
                        psT[:, lc * 128: lc * 128 + w],
                        Eb_sb[:, lc * Ppm + mc * 128: lc * Ppm + (mc + 1) * 128],
                        ident[:, 0:w],
                    )
                if g == 0:
                    return
                nc.vector.tensor_copy(Ea_sb[:, mc * Pls:(mc + 1) * Pls], psT[:, 0:Pls])

            for c in range(PCM):
                transp_half(c, 0)
                psO2 = pso_pool.tile([128, EW], F32, tag="psO")
                for k in range(PCL):
                    nc.tensor.matmul(
                        psO2[:],
                        Eb_sb[:, k * Ppm + c * 128: k * Ppm + (c + 1) * 128],
                        v1e_sb[:, k * EW:(k + 1) * EW],
                        start=(k == 0),
                        stop=(k == PCL - 1),
                    )
                nc.vector.tensor_copy(out2_sb[:, c * EW:(c + 1) * EW], psO2[:])
                if j == len(specs) - 1:
                    # last slot: per-chunk output DMA so the final transfer
                    # (on the kernel-exit critical path) is small
                    nc.gpsimd.dma_start(out2[j][:, c], out2_sb[:, c * EW:(c + 1) * EW])
                transp_half(c, 1)
            if j != len(specs) - 1:
                nc.gpsimd.dma_start(out2[j].rearrange("p c j -> p (c j)"), out2_sb[:, 0:PCM * EW])

            prev = (Ea_sb, v2e_sb, out1T_sb, sp, j)

        final_units = [(dh, lp) for dh in (0, 1) for lp in _lp_pieces(prev[3]["Pls"])]
        for k, u in enumerate(final_units):
            # alternate DMA-issue queues at the tail so the ~0.6us issue
            # costs overlap instead of serializing on one queue
            attend_a_unit(u, prev, dma_eng=(nc.sync if k % 2 else nc.gpsimd))

    nc.compile()
    return nc


def _prep_slot_inputs(sp, v1b, n1b, v2b, n2b):
    """One batch -> the slot's input tensors. v1b/v2b [L, D] f32 full rows;
    n1b/n2b keep counts after compaction (rows [0:n) valid, rest zero)."""
    f32 = np.float32
    bf = ml_dtypes.bfloat16
    Ppl, Ppm, PCL, PCM = sp["Ppl"], sp["Ppm"], sp["PCL"], sp["PCM"]
    v1c = np.zeros((Ppl, D), f32)
    v1c[:n1b] = v1b[:n1b]
    v2c = np.zeros((Ppm, D), f32)
    v2c[:n2b] = v2b[:n2b]
    ones1 = np.zeros((Ppl, 1), f32)
    ones1[:n1b] = 1.0
    zeros = np.zeros((Ppl, 1), f32)
    v1e = np.concatenate([v1c, ones1, zeros], axis=1).reshape(PCL, 128, EW)
    v2e = v2c.reshape(PCM, 128, D)
    return {
        "v1t": np.ascontiguousarray(
            v1c.T.reshape(2, 128, Ppl).transpose(1, 0, 2).reshape(128, 2 * Ppl)
        ).astype(np.float16),
        "v2t": np.ascontiguousarray(
            v2c.T.reshape(2, 128, Ppm).transpose(1, 0, 2).reshape(128, 2 * Ppm)
        ).astype(np.float16),
        "v1e": np.ascontiguousarray(v1e.transpose(1, 0, 2)).astype(bf),
        "v2e": np.ascontiguousarray(v2e.transpose(1, 0, 2)).astype(bf),
    }


def run_on_hw(v1, v1_mask, v2, v2_mask, trace=False, nc=None, plan=None):
    i1s = [np.flatnonzero(~v1_mask[b]) for b in range(B)]
    i2s = [np.flatnonzero(~v2_mask[b]) for b in range(B)]
    n1 = np.array([len(i) for i in i1s])
    n2 = np.array([len(i) for i in i2s])
    if plan is None:
        plan = plan_slots(n1, n2)
    groups, specs = plan
    if nc is None:
        nc = build_nc(specs)
    in_maps = [{"ident": np.eye(128, dtype=ml_dtypes.bfloat16),
                "cbias": np.full((128, 1), -C_SHIFT, np.float32)} for _ in range(NCORES)]
    for j, (g, sp) in enumerate(zip(groups, specs)):
        for i, gb in enumerate(g):
            v1c = v1[gb][i1s[gb]]
            v2c = v2[gb][i2s[gb]]
            m = _prep_slot_inputs(sp, v1c, len(i1s[gb]), v2c, len(i2s[gb]))
            for nm, arr in m.items():
                in_maps[i][f"{nm}{j}"] = arr
    res = run_bass_kernel_spmd(nc, in_maps, core_ids=list(range(NCORES)), trace=trace)
    a1 = np.zeros((B, L, D), np.float32)
    a2 = np.zeros((B, L, D), np.float32)
    for j, (g, sp) in enumerate(zip(groups, specs)):
        Pls, PCM = sp["Pls"], sp["PCM"]
        for i, gb in enumerate(g):
            r = res.results[i]
            n1b, n2b = len(i1s[gb]), len(i2s[gb])
            # o1T [128(dpart), 2*Pls] -> [Pls, 256] unnormalized, / S_a
            u1 = r[f"o1T{j}"].reshape(128, 2, Pls).transpose(2, 1, 0).reshape(Pls, 2 * 128)
            sa = r[f"sa{j}"].T.reshape(-1)  # [Ppl]
            a1[gb, i1s[gb]] = (u1[:n1b].astype(np.float32)
                               / np.maximum(sa[:n1b, None], 1e-30))
            # out2 [128, PCM, EW] -> [Ppm, EW]; col D = S_b
            u2 = r[f"out2_{j}"].transpose(1, 0, 2).reshape(PCM * 128, EW)[:n2b]
            u2 = u2.astype(np.float32)
            a2[gb, i2s[gb]] = u2[:, 0:D] / np.maximum(u2[:, D:D + 1], 1e-30)
    return (a1, a2), res


def kernel(v1, v1_mask, v2, v2_mask):
    v1 = np.asarray(v1, np.float32)
    v2 = np.asarray(v2, np.float32)
    v1_mask = np.asarray(v1_mask)
    v2_mask = np.asarray(v2_mask)
    (a1, a2), _ = run_on_hw(v1, v1_mask, v2, v2_mask, trace=False)
    return a1, a2


# revision 21
# speedup vs baseline: 1.0965x; 1.0034x over previous
"""Bidirectional attention kernel for Trainium2 (8 NeuronCores, data-parallel
over batch) with host-side mask compaction and per-slot adaptive shapes.

~50% of rows on each side are padding (mask True). Masked rows contribute
exp(MASK_FILL - max) ~ 0 to the softmax sums, and their output rows are
zeroed. So we gather the keep rows on the host, and run dense bidirectional
attention on compacted [n1, n2] similarity slabs. Zero-padded rows
self-mask: v=0 -> sim=0 -> exp(0-88) ~ 6e-39 ~ 0 in bf16, and their
ones-column entry is 0 so they don't touch denominators.

The 64 batches are grouped into 8 program slots of 8 (one batch per core
per slot, SPMD) so that batches with the same chunk shape share a slot:
slot j is compiled for (PCL_j, PCM_j) 128-chunks = the max keep counts in
its group. Groups are formed per exact chunk-class to minimize the summed
chunk products (PE work is ~ PCL*PCM).

Math (per slot, compacted):
  sim[l, m] = v1c[l] . v2c[m]                    (fp16 matmuls, [l,m] layout)
  E[l, m]   = exp(sim - C)  (ACT, accum_out -> S_a[l] = sum_m E[l,m])
  Ea[m, l]  = transpose(E)  (PE transpose + DVE copy psum->sbuf)
  u1T[d, l] = sum_m v2e[m, d] Ea[m, l]   (v2-stationary matmuls, PSUM accum
              over m-chunks; drained by plain DVE copies, UNNORMALIZED)
  u2[m, :]  = sum_l E[l, m] v1e[l, :]    (ones col in v1e -> S_b at col D;
              drained UNNORMALIZED by ACT copies)
  host: attended_v1[l] = u1T[:, l] / S_a[l]   (S_a DMA'd out per slot)
        attended_v2[m] = u2[m, 0:D] / u2[m, D]

C is a fixed softmax shift (exp(x-C) instead of exp(x-max)): mathematically
identical softmax, safe because |sim| <~ 91 << 176 and underflow terms are
negligible relative to row sums.
"""
import sys
import types

import ml_dtypes
import numpy as np
from contextlib import ExitStack


def _install_axon_hooks_shim():
    """The image's antenv package lacks the optional axon_hooks module that
    the axon boot shim uses to register the NTFF profiling hook (it degrades
    silently without it). Provide it and redo the registration the boot shim
    skipped, so trace=True works."""
    if "antenv.axon_hooks" not in sys.modules:
        mod = types.ModuleType("antenv.axon_hooks")
        mod._hook = None

        def set_axon_ntff_profile_hook(hook):
            mod._hook = hook

        def get_axon_ntff_profile_hook():
            return mod._hook

        mod.set_axon_ntff_profile_hook = set_axon_ntff_profile_hook
        mod.get_axon_ntff_profile_hook = get_axon_ntff_profile_hook
        sys.modules["antenv.axon_hooks"] = mod
        try:
            import antenv

            antenv.axon_hooks = mod
        except ImportError:
            pass
    mod = sys.modules["antenv.axon_hooks"]
    if getattr(mod, "_hook", None) is None:
        try:
            from trn_agent_boot.trn_boot import _ntff_profile_via_ctypes

            mod._hook = _ntff_profile_via_ctypes("/opt/axon/libaxon_pjrt.so")
        except Exception:
            pass


_install_axon_hooks_shim()

import concourse.bacc as bacc
import concourse.mybir as mybir
import concourse.tile as tile
from concourse.bass_utils import run_bass_kernel_spmd

F32 = mybir.dt.float32
BF16 = mybir.dt.bfloat16
FP16 = mybir.dt.float16
AF = mybir.ActivationFunctionType
ALU = mybir.AluOpType

B, L, D = 64, 1024, 256
NCORES = 8
BPC = B // NCORES          # batches per core == number of program slots
C_SHIFT = np.float32(88.0)
EW = D + 2  # v1e free width: col 256 = ones (S_b denominator), col 257 = 0
# pad (even free-dim count for the ISA)


def _even_ceil(x):
    return int(x + (x & 1))


def plan_slots(n1, n2):
    """Group the 64 batches into BPC slots of NCORES batches with matching
    chunk shapes. Returns (groups, specs): groups[j] = list of NCORES batch
    ids (core i takes groups[j][i]); specs[j] = dict of per-slot shapes."""
    cl = np.maximum(1, -(-n1 // 128)).astype(int)
    cm = np.maximum(1, -(-n2 // 128)).astype(int)
    classes = {}
    for b in range(B):
        classes.setdefault((int(cl[b]), int(cm[b])), []).append(b)
    groups = []
    leftovers = []
    for key in sorted(classes, key=lambda k: (-k[0] * k[1], -k[0])):
        lst = classes[key]
        while len(lst) >= NCORES:
            groups.append(lst[:NCORES])
            lst = lst[NCORES:]
        leftovers.extend(lst)
    leftovers.sort(key=lambda b: (-int(cl[b] * cm[b]), -int(cl[b])))
    while leftovers:
        groups.append(leftovers[:NCORES])
        leftovers = leftovers[NCORES:]
    assert len(groups) == BPC
    # order: second-smallest first (fast pipeline fill), smallest last
    # (short attend-A + output-DMA tail), rest in the middle
    groups.sort(key=lambda g: max(cl[b] * cm[b] for b in g))
    groups = [groups[1]] + groups[2:] + [groups[0]]
    specs = []
    for g in groups:
        PCL = int(max(cl[b] for b in g))
        PCM = int(max(cm[b] for b in g))
        mx1 = int(max(n1[b] for b in g))
        mx2 = int(max(n2[b] for b in g))
        specs.append(dict(
            PCL=PCL, PCM=PCM,
            Ppl=PCL * 128, Ppm=PCM * 128,
            Pms=min(PCM * 128, _even_ceil(mx2)),   # sim/exp m free width
            Pls=min(PCL * 128, _even_ceil(mx1)),   # attend-A l free width
            # transpose out width for the last l-chunk
            wl=min(128, _even_ceil(mx1 - (PCL - 1) * 128)),
        ))
    return groups, specs


def _lp_pieces(Pls):
    """Split [0, Pls) into <=512-wide pieces (one PSUM bank each). Balanced
    halves when a split is needed, so no piece is so narrow that its matmuls
    fall under the ~107ns LDWEIGHTS floor."""
    if Pls <= 512:
        return [(0, Pls)]
    h = _even_ceil((Pls + 1) // 2)
    return [(0, h), (h, Pls - h)]


def build_nc(specs):
    nc = bacc.Bacc("TRN2", target_bir_lowering=False, debug=False)

    v1t, v2t, v1e, v2e, o1T, out2, sa_d = [], [], [], [], [], [], []
    for j, sp in enumerate(specs):
        Ppl, Ppm, PCL, PCM, Pls = sp["Ppl"], sp["Ppm"], sp["PCL"], sp["PCM"], sp["Pls"]
        v1t.append(nc.dram_tensor(f"v1t{j}", [128, 2 * Ppl], FP16, kind="ExternalInput").ap())
        v2t.append(nc.dram_tensor(f"v2t{j}", [128, 2 * Ppm], FP16, kind="ExternalInput").ap())
        v1e.append(nc.dram_tensor(f"v1e{j}", [128, PCL, EW], BF16, kind="ExternalInput").ap())
        v2e.append(nc.dram_tensor(f"v2e{j}", [128, PCM, D], BF16, kind="ExternalInput").ap())
        o1T.append(nc.dram_tensor(f"o1T{j}", [128, 2 * Pls], BF16, kind="ExternalOutput").ap())
        out2.append(nc.dram_tensor(f"out2_{j}", [128, PCM, EW], BF16, kind="ExternalOutput").ap())
        sa_d.append(nc.dram_tensor(f"sa{j}", [128, PCL], F32, kind="ExternalOutput").ap())

    with tile.TileContext(nc) as tc, ExitStack() as ctx:
        const_pool = ctx.enter_context(tc.tile_pool(name="const", bufs=1))
        # bufs=3 so slot j+1's input DMAs can be issued at the start of slot
        # j's body without the Sync queue blocking on the j-1 tile recycle.
        in_pool = ctx.enter_context(tc.tile_pool(name="inp", bufs=3))
        e_pool = ctx.enter_context(tc.tile_pool(name="epool", bufs=1))
        ebpool = ctx.enter_context(tc.tile_pool(name="ebpool", bufs=2))
        out_pool = ctx.enter_context(tc.tile_pool(name="outp", bufs=2))
        sa_pool = ctx.enter_context(tc.tile_pool(name="sa", bufs=2))
        psb_pool = ctx.enter_context(tc.tile_pool(name="psb", bufs=2, space="PSUM"))
        # psT (transpose, bf16 <=1280B) and psA (attend-A accum, f32 <=2048B)
        # share one pool: their live ranges are phase-disjoint within a slot.
        pst_pool = ctx.enter_context(tc.tile_pool(name="pst", bufs=2, space="PSUM"))
        pso_pool = ctx.enter_context(tc.tile_pool(name="pso", bufs=2, space="PSUM"))

        # identity built on-chip (gpsimd memset + affine_select): no DMA, so
        # the PE warmup isn't starved behind the input-DMA flood
        ident = const_pool.tile([128, 128], BF16)
        nc.gpsimd.memset(ident[:], 0.0)
        nc.gpsimd.affine_select(
            out=ident[:], in_=ident[:], compare_op=ALU.not_equal, fill=1.0,
            base=0, pattern=[[-1, 128]], channel_multiplier=1,
        )

        # The Pms trim leaves columns [Pms, Ppm) of each Eb row-chunk
        # unwritten; the transposes feed those cells into the attend-A
        # contraction where a NaN bit-pattern from uninitialized SBUF would
        # poison valid rows (0 * NaN = NaN). Zero the full region once:
        # every later write to it is finite exp output.
        eb_max = max(sp["PCL"] * sp["Ppm"] for sp in specs)
        for _ in range(2):
            eb0 = ebpool.tile([128, eb_max], BF16, tag="Eb")
            nc.gpsimd.memset(eb0[:], 0.0)
        cbias = const_pool.tile([128, 1], F32)
        nc.gpsimd.memset(cbias[:], -float(C_SHIFT))

        # Dummy exp so the ACT engine's Exp table load (~1.3us) happens
        # during the initial input-DMA wait instead of right before the
        # first real exp on the critical path.
        scratch = const_pool.tile([128, 2], F32)
        nc.scalar.activation(scratch[:], ident[:, 0:2], AF.Exp,
                             bias=cbias[:], scale=1.0)

        # PE warmup: dummy transposes while the first slot's input DMAs
        # stream in, so the HAM clock-gate is at 2.4 GHz when real matmuls
        # start.
        warm = pst_pool.tile([128, 512], BF16, tag="psT")
        for w in range(32):
            nc.tensor.transpose(warm[:, (w % 4) * 128:(w % 4 + 1) * 128], ident[:], ident[:])

        # attend-A of slot j-1 is interleaved into slot j's sim phase so sim
        # matmuls absorb the psA-recycle latency between attend-A units.
        prev = None  # (Ea_sb, v2e_sb, out1T_sb, spec, j-1)

        def attend_a_unit(u, st, dma_eng=None):
            """One (d-half, l-piece) of attend-A for slot jp: accumulate
            over all m-chunks into a 1-bank PSUM tile, then plain-copy out
            (unnormalized; host divides by S_a)."""
            dh, (lp0, lpw) = u
            Ea_p, v2e_p, out1T_p, spp, jp = st
            Pls_p = spp["Pls"]
            psA = pst_pool.tile([128, lpw], F32, tag="psT")
            for mc in range(spp["PCM"]):
                nc.tensor.matmul(
                    psA[:],
                    v2e_p[:, mc * D + dh * 128: mc * D + (dh + 1) * 128],
                    Ea_p[:, mc * Pls_p + lp0: mc * Pls_p + lp0 + lpw],
                    start=(mc == 0),
                    stop=(mc == spp["PCM"] - 1),
                )
            dst0 = dh * Pls_p + lp0
            nc.vector.tensor_copy(out1T_p[:, dst0:dst0 + lpw], psA[:])
            # output DMA starts go on the (idle) GpSimd queue so the Sync
            # queue never head-of-line blocks input prefetch behind them
            eng = dma_eng if dma_eng is not None else nc.gpsimd
            eng.dma_start(o1T[jp][:, dst0:dst0 + lpw], out1T_p[:, dst0:dst0 + lpw])

        def issue_inputs(j):
            sp = specs[j]
            Ppl, Ppm, PCL, PCM = sp["Ppl"], sp["Ppm"], sp["PCL"], sp["PCM"]
            v1t_sb = in_pool.tile([128, 2 * Ppl], FP16, tag="v1t")
            v2t_sb = in_pool.tile([128, 2 * Ppm], FP16, tag="v2t")
            v1e_sb = in_pool.tile([128, PCL * EW], BF16, tag="v1e")
            v2e_sb = in_pool.tile([128, PCM * D], BF16, tag="v2e")
            if j == 0:
                # fine-grained pieces ordered by first use, so the first sim
                # matmul's operands (v1t k0 chunk 0 + v2t k0 first half)
                # arrive ~2.5us before the full input set.
                h = min(512, Ppm)
                nc.sync.dma_start(v1t_sb[:, 0:128], v1t[j][:, 0:128])
                nc.sync.dma_start(v2t_sb[:, 0:h], v2t[j][:, 0:h])
                nc.sync.dma_start(v1t_sb[:, 128:Ppl], v1t[j][:, 128:Ppl])
                if h < Ppm:
                    nc.sync.dma_start(v2t_sb[:, h:Ppm], v2t[j][:, h:Ppm])
                nc.sync.dma_start(v1t_sb[:, Ppl:2 * Ppl], v1t[j][:, Ppl:2 * Ppl])
                nc.sync.dma_start(v2t_sb[:, Ppm:2 * Ppm], v2t[j][:, Ppm:2 * Ppm])
            else:
                # halves ordered so the k=0 operands (first sim MMs) land first
                nc.sync.dma_start(v1t_sb[:, 0:Ppl], v1t[j][:, 0:Ppl])
                nc.sync.dma_start(v2t_sb[:, 0:Ppm], v2t[j][:, 0:Ppm])
                nc.sync.dma_start(v1t_sb[:, Ppl:2 * Ppl], v1t[j][:, Ppl:2 * Ppl])
                nc.sync.dma_start(v2t_sb[:, Ppm:2 * Ppm], v2t[j][:, Ppm:2 * Ppm])
            nc.sync.dma_start(v1e_sb[:, 0:PCL * EW], v1e[j].rearrange("p c j -> p (c j)"))
            nc.sync.dma_start(v2e_sb[:, 0:PCM * D], v2e[j].rearrange("p c j -> p (c j)"))
            return v1t_sb, v2t_sb, v1e_sb, v2e_sb

        in_tiles = issue_inputs(0)
        for j, sp in enumerate(specs):
            PCL, PCM = sp["PCL"], sp["PCM"]
            Ppl, Ppm, Pms, Pls, wl = sp["Ppl"], sp["Ppm"], sp["Pms"], sp["Pls"], sp["wl"]

            v1t_sb, v2t_sb, v1e_sb, v2e_sb = in_tiles
            if j + 1 < len(specs):
                in_tiles = issue_inputs(j + 1)

            Eb_sb = ebpool.tile([128, PCL * Ppm], BF16, tag="Eb")
            Ea_sb = e_pool.tile([128, PCM * Pls], BF16, tag="Ea")
            Sa_sb = sa_pool.tile([128, PCL], F32, tag="Sa")
            out1T_sb = out_pool.tile([128, 2 * Pls], BF16, tag="o1")
            out2_sb = out_pool.tile([128, PCM * EW], BF16, tag="o2")

            # ---- Phase 1: sim in [l, m] layout + fused shift/exp -> Eb
            # (accum_out gives S_a row sums for free; attend-A units of
            # slot j-1 interleaved between sim groups). Columns beyond Pms
            # of each Eb row-chunk are stale: they only feed discarded
            # output rows (m >= n2) or get multiplied by zero pad rows.
            units = [] if prev is None else [
                (dh, lp) for dh in (0, 1) for lp in _lp_pieces(prev[3]["Pls"])
            ]
            n_inter = min(len(units), max(PCL - 1, 0))
            ui = 0
            for lc in range(PCL):
                psB = psb_pool.tile([128, Ppm], F32, tag="psB")
                for k in range(2):
                    for mh in range(0, Pms, 512):
                        mw = min(512, Pms - mh)
                        nc.tensor.matmul(
                            psB[:, mh:mh + mw],
                            v1t_sb[:, k * Ppl + lc * 128: k * Ppl + (lc + 1) * 128],
                            v2t_sb[:, k * Ppm + mh: k * Ppm + mh + mw],
                            start=(k == 0),
                            stop=(k == 1),
                        )
                nc.scalar.activation(
                    Eb_sb[:, lc * Ppm: lc * Ppm + Pms],
                    psB[:, 0:Pms],
                    AF.Exp,
                    bias=cbias[:],
                    scale=1.0,
                    accum_out=Sa_sb[:, lc:lc + 1],
                )
                if prev is not None and 1 <= lc and ui < n_inter:
                    attend_a_unit(units[ui], prev)
                    ui += 1

            nc.gpsimd.dma_start(sa_d[j][:], Sa_sb[:, 0:PCL])
            while ui < len(units):
                attend_a_unit(units[ui], prev)
                ui += 1

            # ---- Phase 2+3 interleaved: attend-B groups between transpose
            # groups so the PSUM->SBUF copies (DVE) overlap attend-B matmul
            # time on the PE instead of gating it. attend-B output (with the
            # S_b ones-column) is drained unnormalized by ACT copies.
            psT_cur = [None]
            g0 = (PCL + 1) // 2  # transposes emitted before the attend-B group

            def transp_half(mc, g):
                if g == 0:
                    psT_new = pst_pool.tile([128, Ppl], BF16, tag="psT")
                    psT_cur[0] = psT_new
                psT = psT_cur[0]
                lcs = range(g0) if g == 0 else range(g0, PCL)
                for lc in lcs:
                    w = wl if lc == PCL - 1 else 128
                    nc.tensor.transpose(
                        psT[:, lc * 128: lc * 128 + w],
                        Eb_sb[:, lc * Ppm + mc * 128: lc * Ppm + (mc + 1) * 128],
                        ident[:, 0:w],
                    )
                if g == 0:
                    return
                nc.vector.tensor_copy(Ea_sb[:, mc * Pls:(mc + 1) * Pls], psT[:, 0:Pls])

            for c in range(PCM):
                transp_half(c, 0)
                psO2 = pso_pool.tile([128, EW], F32, tag="psO")
                for k in range(PCL):
                    nc.tensor.matmul(
                        psO2[:],
                        Eb_sb[:, k * Ppm + c * 128: k * Ppm + (c + 1) * 128],
                        v1e_sb[:, k * EW:(k + 1) * EW],
                        start=(k == 0),
                        stop=(k == PCL - 1),
                    )
                nc.vector.tensor_copy(out2_sb[:, c * EW:(c + 1) * EW], psO2[:])
                if j == len(specs) - 1:
                    # last slot: per-chunk output DMA so the final transfer
                    # (on the kernel-exit critical path) is small
                    nc.gpsimd.dma_start(out2[j][:, c], out2_sb[:, c * EW:(c + 1) * EW])
                transp_half(c, 1)
            if j != len(specs) - 1:
                nc.gpsimd.dma_start(out2[j].rearrange("p c j -> p (c j)"), out2_sb[:, 0:PCM * EW])

            prev = (Ea_sb, v2e_sb, out1T_sb, sp, j)

        final_units = [(dh, lp) for dh in (0, 1) for lp in _lp_pieces(prev[3]["Pls"])]
        for k, u in enumerate(final_units):
            # alternate DMA-issue queues at the tail so the ~0.6us issue
            # costs overlap instead of serializing on one queue
            attend_a_unit(u, prev, dma_eng=(nc.sync if k % 2 else nc.gpsimd))

    nc.compile()
    return nc


def _prep_slot_inputs(sp, v1b, n1b, v2b, n2b):
    """One batch -> the slot's input tensors. v1b/v2b [L, D] f32 full rows;
    n1b/n2b keep counts after compaction (rows [0:n) valid, rest zero)."""
    f32 = np.float32
    bf = ml_dtypes.bfloat16
    Ppl, Ppm, PCL, PCM = sp["Ppl"], sp["Ppm"], sp["PCL"], sp["PCM"]
    v1c = np.zeros((Ppl, D), f32)
    v1c[:n1b] = v1b[:n1b]
    v2c = np.zeros((Ppm, D), f32)
    v2c[:n2b] = v2b[:n2b]
    ones1 = np.zeros((Ppl, 1), f32)
    ones1[:n1b] = 1.0
    zeros = np.zeros((Ppl, 1), f32)
    v1e = np.concatenate([v1c, ones1, zeros], axis=1).reshape(PCL, 128, EW)
    v2e = v2c.reshape(PCM, 128, D)
    return {
        "v1t": np.ascontiguousarray(
            v1c.T.reshape(2, 128, Ppl).transpose(1, 0, 2).reshape(128, 2 * Ppl)
        ).astype(np.float16),
        "v2t": np.ascontiguousarray(
            v2c.T.reshape(2, 128, Ppm).transpose(1, 0, 2).reshape(128, 2 * Ppm)
        ).astype(np.float16),
        "v1e": np.ascontiguousarray(v1e.transpose(1, 0, 2)).astype(bf),
        "v2e": np.ascontiguousarray(v2e.transpose(1, 0, 2)).astype(bf),
    }


def run_on_hw(v1, v1_mask, v2, v2_mask, trace=False, nc=None, plan=None):
    i1s = [np.flatnonzero(~v1_mask[b]) for b in range(B)]
    i2s = [np.flatnonzero(~v2_mask[b]) for b in range(B)]
    n1 = np.array([len(i) for i in i1s])
    n2 = np.array([len(i) for i in i2s])
    if plan is None:
        plan = plan_slots(n1, n2)
    groups, specs = plan
    if nc is None:
        nc = build_nc(specs)
    in_maps = [{} for _ in range(NCORES)]
    for j, (g, sp) in enumerate(zip(groups, specs)):
        for i, gb in enumerate(g):
            v1c = v1[gb][i1s[gb]]
            v2c = v2[gb][i2s[gb]]
            m = _prep_slot_inputs(sp, v1c, len(i1s[gb]), v2c, len(i2s[gb]))
            for nm, arr in m.items():
                in_maps[i][f"{nm}{j}"] = arr
    res = run_bass_kernel_spmd(nc, in_maps, core_ids=list(range(NCORES)), trace=trace)
    a1 = np.zeros((B, L, D), np.float32)
    a2 = np.zeros((B, L, D), np.float32)
    for j, (g, sp) in enumerate(zip(groups, specs)):
        Pls, PCM = sp["Pls"], sp["PCM"]
        for i, gb in enumerate(g):
            r = res.results[i]
            n1b, n2b = len(i1s[gb]), len(i2s[gb])
            # o1T [128(dpart), 2*Pls] -> [Pls, 256] unnormalized, / S_a
            u1 = r[f"o1T{j}"].reshape(128, 2, Pls).transpose(2, 1, 0).reshape(Pls, 2 * 128)
            sa = r[f"sa{j}"].T.reshape(-1)  # [Ppl]
            a1[gb, i1s[gb]] = (u1[:n1b].astype(np.float32)
                               / np.maximum(sa[:n1b, None], 1e-30))
            # out2 [128, PCM, EW] -> [Ppm, EW]; col D = S_b
            u2 = r[f"out2_{j}"].transpose(1, 0, 2).reshape(PCM * 128, EW)[:n2b]
            u2 = u2.astype(np.float32)
            a2[gb, i2s[gb]] = u2[:, 0:D] / np.maximum(u2[:, D:D + 1], 1e-30)
    return (a1, a2), res


def kernel(v1, v1_mask, v2, v2_mask):
    v1 = np.asarray(v1, np.float32)
    v2 = np.asarray(v2, np.float32)
    v1_mask = np.asarray(v1_mask)
    v2_mask = np.asarray(v2_mask)
    (a1, a2), _ = run_on_hw(v1, v1_mask, v2, v2_mask, trace=False)
    return a1, a2
